# revision 22
# baseline (speedup 1.0000x reference)
"""ALIGNN (nn_ALIGNN_PyG) distributed Trainium2 Bass kernel, 8 NeuronCores.

Sharding (graph-data parallel, comm-minimized):
  - e-rows (line-graph nodes, E) sharded contiguously: E/8 rows per core.
  - Triplets assigned to the owner of their dst edge, processed in dst-sorted
    order, packed into 128-row tiles aligned to segment boundaries (host-side
    padding) so per-tile selection-matmul segment sums never straddle a tile
    and each output row is written by exactly one scatter descriptor.
  - Edge gates need e[src] rows owned by other cores -> AllToAll of the
    unique requested rows (host-computed routing tables).
  - Node-graph edges processed at the core owning the e-row; per-core partial
    aggregates over all N nodes are AllReduced in 4 row-chunks; the node
    post-phase is computed replicated so x stays replicated on all cores.
  - bf16 storage/streams, fp32 PSUM/LN statistics.

All index manipulation (sorting, routing, padding) happens on the host in
numpy; all floating-point math runs on device.
"""

import sys

sys.path.insert(0, "/opt/trn_rl_repo")

import numpy as np
import ml_dtypes

from concourse import bass, bacc, mybir, tile
from concourse.bass import ds, IndirectOffsetOnAxis
from concourse.masks import make_identity

F32 = mybir.dt.float32
BF16 = mybir.dt.bfloat16
I32 = mybir.dt.int32

P = 128
HID = 64
ATOM = 92
EBINS = 40
TBINS = 20
RADIUS = 10.0
NLAY = 4
OOB = 1 << 30
PADSEG = 300.0
EPS = 1e-5
AX = mybir.AxisListType
AF = mybir.ActivationFunctionType
OP = mybir.AluOpType

BF = ml_dtypes.bfloat16


def _bf(x):
    return np.ascontiguousarray(np.asarray(x, dtype=np.float32)).astype(BF)


def _f32(x):
    return np.ascontiguousarray(np.asarray(x, dtype=np.float32))


def _rup(x, m):
    return ((x + m - 1) // m) * m


# ----------------------------------------------------------------------------
# Host preprocessing
# ----------------------------------------------------------------------------

def _segment_slots(d, PP=P):
    """d: sorted int array. Greedy-pack runs of equal values into PP-row tiles
    so no run straddles a tile boundary. Returns slots, per-element tile-local
    run labels (0..PP-1, the run's first slot within its tile; small ints so
    the PE-matmul broadcast of labels is exact even at reduced precision),
    and the tile count."""
    n = len(d)
    if n == 0:
        return np.zeros(0, np.int64), np.zeros(0, np.int64), 0
    bnd = np.flatnonzero(np.diff(d)) + 1
    starts = np.concatenate([[0], bnd]).tolist()
    ends = np.concatenate([bnd, [n]]).tolist()
    slot = np.empty(n, np.int64)
    lab = np.empty(n, np.int64)
    pos = 0
    ar = np.arange(PP)
    for s, e in zip(starts, ends):
        ln = e - s
        assert ln <= PP, f"segment run {ln} > {PP}"
        if (pos % PP) + ln > PP:
            pos = ((pos // PP) + 1) * PP
        slot[s:e] = ar[:ln] + pos
        lab[s:e] = pos % PP
        pos += ln
    return slot, lab, (pos + PP - 1) // PP


def _col128(x):
    """[L] stream -> [128, L//128] with column j = tile j."""
    return np.ascontiguousarray(x.reshape(-1, P).T)


def _pack_stream(dl, extras, dummy):
    """dl: sorted local dst ids. extras: {name: (aligned_array, pad_value)}.
    Masked (pad / non-first-of-segment) rows scatter to the dummy row."""
    slot, lab, nt = _segment_slots(dl)
    L = nt * P
    out = {}
    gd = np.zeros(L, np.int64)
    gd[slot] = dl
    sd = np.full(L, PADSEG, np.float32)
    sd[slot] = lab.astype(np.float32)
    sc = np.full(L, dummy, np.int64)
    if len(dl):
        first = np.ones(len(dl), bool)
        first[1:] = dl[1:] != dl[:-1]
        sc[slot[first]] = dl[first]
    out["gd"], out["sd"], out["sc"] = gd, sd, sc
    for k, (arr, padv) in extras.items():
        a = np.full(L, padv, arr.dtype)
        a[slot] = arr
        out[k] = a
    out["n"] = L
    return out


def _pad_to(st, L, pads):
    for k, padv in pads.items():
        a = st[k]
        if len(a) < L:
            st[k] = np.concatenate([a, np.full(L - len(a), padv, a.dtype)])
    st["n"] = L


_BASE_PADS = {"gd": np.int64(0), "sd": np.float32(PADSEG)}


def prep(inputs, C=8, BLK=4096):
    x_atom = _f32(inputs["x_atom"])
    edge_dist = _f32(inputs["edge_dist"])
    angle_cos = _f32(inputs["angle_cos"])
    params = inputs["params"]
    ei = np.asarray(inputs["edge_index"]).astype(np.int64)
    lg = np.asarray(inputs["lg_edge_index"]).astype(np.int64)
    batch = np.asarray(inputs["batch"]).astype(np.int64)
    G = int(np.asarray(inputs["num_graphs"]))

    N, E = x_atom.shape[0], edge_dist.shape[0]
    assert E % C == 0
    ES = E // C
    NP_ = _rup(N, BLK)
    ESP = _rup(ES, BLK)
    NQ = 4
    assert N % NQ == 0
    NQR = N // NQ

    meta = dict(C=C, N=N, E=E, G=G, ES=ES, NP=NP_, ESP=ESP, NQ=NQ, NQR=NQR,
                BLK=BLK)

    # ---- triplet (edge-EGC) streams ----------------------------------------
    src_t, dst_t = lg[0], lg[1]
    own_t = dst_t // ES
    trip = []
    for c in range(C):
        m = np.flatnonzero(own_t == c)
        o = m[np.argsort(dst_t[m], kind="stable")]
        dl = dst_t[o] - c * ES
        trip.append(_pack_stream(dl, {"sg": (src_t[o], np.int64(-1)),
                                      "ang": (angle_cos[o], np.float32(0))},
                                 ESP))
    LT = _rup(max(t["n"] for t in trip), BLK)
    for t in trip:
        _pad_to(t, LT, dict(_BASE_PADS, sc=np.int64(ESP), sg=np.int64(-1),
                            ang=np.float32(0)))
    meta["LT"] = LT

    # ---- AllToAll routing for e[src] rows ----------------------------------
    uniq = [[None] * C for _ in range(C)]
    for c in range(C):
        sgl = trip[c]["sg"]
        for s in range(C):
            sel = sgl[(sgl >= s * ES) & (sgl < (s + 1) * ES)]
            uniq[s][c] = np.unique(sel)
    PADM = max(max(len(uniq[s][c]) for c in range(C)) for s in range(C))
    PADM = max(PADM, 1)
    PADM = _rup(PADM, BLK // C) if (BLK % C == 0) else _rup(PADM, P)
    while (C * PADM) % BLK != 0:
        PADM += P
    meta["PADM"] = PADM
    send_idx = np.zeros((C, C * PADM), np.int64)
    for s in range(C):
        for c in range(C):
            ids = uniq[s][c] - s * ES
            send_idx[s, c * PADM:c * PADM + len(ids)] = ids
    for c in range(C):
        sgl = trip[c]["sg"]
        gs = np.zeros(LT, np.int64)
        for s in range(C):
            msk = (sgl >= s * ES) & (sgl < (s + 1) * ES)
            gs[msk] = s * PADM + np.searchsorted(uniq[s][c], sgl[msk])
        gs[sgl < 0] = 0
        trip[c]["gs"] = gs

    # ---- node-EGC streams, quartered for chunked AllReduce -----------------
    src_n, dst_n = ei[0], ei[1]
    node = []
    for c in range(C):
        j0, j1 = c * ES, (c + 1) * ES
        dd = dst_n[j0:j1]
        ss = src_n[j0:j1]
        qs = []
        for q in range(NQ):
            m = np.flatnonzero((dd >= q * NQR) & (dd < (q + 1) * NQR))
            o = m[np.argsort(dd[m], kind="stable")]
            dl = dd[o] - q * NQR
            qs.append(_pack_stream(dl, {"ge": (o.astype(np.int64), np.int64(0)),
                                        "gxs": (ss[o], np.int64(0)),
                                        "gxd": (dd[o], np.int64(0))}, NQR))
        node.append(qs)
    LNQ = [_rup(max(max(node[c][q]["n"] for c in range(C)), BLK), BLK)
           for q in range(NQ)]
    for c in range(C):
        for q in range(NQ):
            _pad_to(node[c][q], LNQ[q],
                    dict(_BASE_PADS, sc=np.int64(NQR), ge=np.int64(0),
                         gxs=np.int64(0), gxd=np.int64(0)))
    meta["LNQ"] = LNQ

    # ---- weights -----------------------------------------------------------
    def lin(p):
        return _f32(p["w"]), _f32(p["b"])

    egcs = []
    for l in range(NLAY):
        egcs.append(params["alignn"][l]["edge"])
        egcs.append(params["alignn"][l]["node"])
    egcs.extend(params["gcn"])
    NE = len(egcs)
    meta["NE"] = NE

    Wsd = np.zeros((P, NE * HID), np.float32)
    Weg = np.zeros((HID, NE * HID), np.float32)
    Wdu = np.zeros((P, NE * HID), np.float32)
    Wsu = np.zeros((HID, NE * HID), np.float32)
    bgate = np.zeros((HID, NE), np.float32)
    bdu = np.zeros((HID, NE), np.float32)
    bsu = np.zeros((HID, NE), np.float32)
    lngr = np.zeros((NE + 3, HID), np.float32)
    lnbr = np.zeros((NE + 3, HID), np.float32)
    for i, p in enumerate(egcs):
        sw, sb_ = lin(p["sg"])
        dw, db = lin(p["dg"])
        ew, eb = lin(p["eg"])
        uw, ub = lin(p["du"])
        tw, tb = lin(p["su"])
        Wsd[:HID, i * HID:(i + 1) * HID] = sw
        Wsd[HID:, i * HID:(i + 1) * HID] = dw
        Weg[:, i * HID:(i + 1) * HID] = ew
        Wdu[HID:, i * HID:(i + 1) * HID] = uw
        Wsu[:, i * HID:(i + 1) * HID] = tw
        bgate[:, i] = sb_ + db + eb
        bdu[:, i] = ub
        bsu[:, i] = tb
        lngr[i] = _f32(p["ln_g"])
        lnbr[i] = _f32(p["ln_b"])

    embs = [params["atom_emb"], params["edge_emb"], params["angle_emb"]]
    Wat = np.zeros((ATOM, HID), np.float32)
    Wed = np.zeros((EBINS, HID), np.float32)
    Wan = np.zeros((TBINS, HID), np.float32)
    bemb = np.zeros((HID, 3), np.float32)
    for i, p in enumerate(embs):
        w, b = lin(p)
        [Wat, Wed, Wan][i][:, :] = w
        bemb[:, i] = b
        lngr[NE + i] = _f32(p["ln_g"])
        lnbr[NE + i] = _f32(p["ln_b"])

    Wfc, bfc = lin(params["fc"])
    Wout, bout = lin(params["out"])

    cent_e = np.linspace(0.0, RADIUS, EBINS).astype(np.float32)
    gam_e = 1.0 / (cent_e[1] - cent_e[0]) ** 2
    cent_a = np.linspace(-1.0, 1.0, TBINS).astype(np.float32)
    gam_a = 1.0 / (cent_a[1] - cent_a[0]) ** 2
    meta["gam_e"], meta["gam_a"] = float(gam_e), float(gam_a)
    meta["bout"] = float(bout[0])
    meta["GH"] = _rup(G, P) // P

    xa_pad = np.zeros((NP_, ATOM), np.float32)
    xa_pad[:N] = x_atom
    bat_pad = np.full(NP_, 2.0e6, np.float32)
    bat_pad[:N] = batch.astype(np.float32)

    shared = dict(
        x_atom=xa_pad,
        batchf=_col128(bat_pad),
        Wsd=_bf(Wsd), Weg=_bf(Weg), Wdu=_bf(Wdu), Wsu=_bf(Wsu),
        bgate=bgate, bdu=bdu, bsu=bsu,
        lngr=lngr, lnbr=lnbr,
        lngc=np.ascontiguousarray(lngr.T), lnbc=np.ascontiguousarray(lnbr.T),
        Wat=_bf(Wat), Wed=_bf(Wed), Wan=_bf(Wan), bemb=bemb,
        Wfc=_bf(Wfc), bfc=_f32(bfc).reshape(HID, 1),
        Wout=_bf(Wout).reshape(HID, 1),
        cent_e=cent_e.reshape(EBINS, 1), cent_a=cent_a.reshape(TBINS, 1),
    )

    in_maps = []
    for c in range(C):
        ed_pad = np.zeros(ESP, np.float32)
        ed_pad[:ES] = edge_dist[c * ES:(c + 1) * ES]
        t = trip[c]
        m = dict(shared)
        m["edist"] = ed_pad
        m["angp"] = t["ang"]
        m["t_gd"] = _col128(t["gd"].astype(np.int32))
        m["t_gs"] = _col128(t["gs"].astype(np.int32))
        m["t_sc"] = _col128(t["sc"].astype(np.int32))
        m["t_sd"] = _col128(t["sd"])
        m["t_sdT"] = t["sd"].reshape(1, -1)
        m["sendix"] = _col128(send_idx[c].astype(np.int32))
        for q in range(NQ):
            nq = node[c][q]
            m[f"n_ge{q}"] = _col128(nq["ge"].astype(np.int32))
            m[f"n_gxs{q}"] = _col128(nq["gxs"].astype(np.int32))
            m[f"n_gxd{q}"] = _col128(nq["gxd"].astype(np.int32))
            m[f"n_sc{q}"] = _col128(nq["sc"].astype(np.int32))
            m[f"n_sd{q}"] = _col128(nq["sd"])
            m[f"n_sdT{q}"] = nq["sd"].reshape(1, -1)
        in_maps.append(m)

    return meta, in_maps


# ----------------------------------------------------------------------------
# Device kernel
# ----------------------------------------------------------------------------

def _bcast_mid(ap2d, nsub, inner):
    """[128, k] AP -> [128, (1,k)... wait: build [p, nsub, inner] view with the
    given free pattern pairs."""
    return bass.AP(ap2d.tensor, ap2d.offset, [ap2d.ap[0], (1, nsub),
                                              (0, inner)])


def _bcast_row(ap2d, nsub, inner):
    """[128, inner] AP -> [p, nsub(bcast), inner]."""
    return bass.AP(ap2d.tensor, ap2d.offset, [ap2d.ap[0], (0, nsub),
                                              (1, inner)])


class Consts:
    pass


def _load_consts(tc, nc, ins, meta):
    K = Consts()
    cp = tc.alloc_tile_pool(name="consts", bufs=1)
    K.pool = cp

    def sb(name):
        a = ins[name]
        t = cp.tile(list(a.shape), a.dtype, name="c_" + name)
        nc.sync.dma_start(out=t[:], in_=a[:])
        return t

    for nm in ["Wsd", "Weg", "Wdu", "Wsu", "bgate", "bdu", "bsu",
               "Wat", "Wed", "Wan", "bemb", "Wfc", "bfc", "Wout",
               "cent_e", "cent_a", "lngc", "lnbc"]:
        setattr(K, nm, sb(nm))

    NE = meta["NE"]
    K.lng = []
    K.lnb = []
    for i in range(NE):
        gr = cp.tile([1, HID], F32, name=f"lngr{i}")
        nc.sync.dma_start(out=gr[:], in_=ins["lngr"][i:i + 1, :])
        br = cp.tile([1, HID], F32, name=f"lnbr{i}")
        nc.sync.dma_start(out=br[:], in_=ins["lnbr"][i:i + 1, :])
        g = cp.tile([P, HID], F32, name=f"lng{i}")
        b = cp.tile([P, HID], F32, name=f"lnb{i}")
        nc.gpsimd.partition_broadcast(g[:], gr[:])
        nc.gpsimd.partition_broadcast(b[:], br[:])
        K.lng.append(g)
        K.lnb.append(b)

    K.ident = cp.tile([P, P], BF16, name="identbf")
    make_identity(nc, K.ident[:])
    K.identf = cp.tile([P, P], F32, name="identf")
    make_identity(nc, K.identf[:])

    K.ones1 = cp.tile([1, P], F32, name="ones1")
    nc.gpsimd.memset(K.ones1[:], 1.0)

    K.stS = cp.tile([P, 1], BF16, name="stS")
    nc.gpsimd.memset(K.stS[:], 0.0)
    nc.gpsimd.memset(K.stS[:HID, 0:1], 1.0)
    K.stQ = cp.tile([P, 1], BF16, name="stQ")
    nc.gpsimd.memset(K.stQ[:], 0.0)
    nc.gpsimd.memset(K.stQ[HID:, 0:1], 1.0)

    K.epsP = cp.tile([P, 1], F32, name="epsP")
    nc.gpsimd.memset(K.epsP[:], EPS)
    K.boutP = cp.tile([P, 1], F32, name="boutP")
    nc.gpsimd.memset(K.boutP[:], float(meta["bout"]))

    GH = meta["GH"]
    it = cp.tile([P, GH * P], I32, name="iotai")
    nc.gpsimd.iota(it[:], pattern=[[1, GH * P]], base=0, channel_multiplier=0)
    K.iotaf = cp.tile([P, GH * P], F32, name="iotaf")
    nc.vector.tensor_copy(K.iotaf[:], it[:])
    return K


def build(tc, outs, ins, meta):
    nc = tc.nc
    C, BLK = meta["C"], meta["BLK"]
    ES, ESP, NP_, LT = meta["ES"], meta["ESP"], meta["NP"], meta["LT"]
    NQ, NQR, LNQ = meta["NQ"], meta["NQR"], meta["LNQ"]
    PADM, NE, G, GH = meta["PADM"], meta["NE"], meta["G"], meta["GH"]
    N = meta["N"]
    SUB = BLK // P
    GRP = BLK // 512
    RG = [list(range(C))]

    K = _load_consts(tc, nc, ins, meta)

    dram = tc.alloc_tile_pool(name="dram", bufs=1, space="DRAM")
    e_bufs = [dram.tile([ESP, HID], BF16, name=f"e{l}")
              for l in range(NLAY + 1)]
    x_bufs = [dram.tile([NP_, HID], BF16, name=f"x{l}")
              for l in range(2 * NLAY + 1)]
    aT = dram.tile([HID, LT], BF16, name="aT")
    send_b = [dram.tile([C * PADM, HID], BF16, name=f"send{l}")
              for l in range(NLAY)]
    recv_b = [dram.tile([C * PADM, HID], BF16, name=f"recv{l}")
              for l in range(NLAY)]
    aggr_e = [dram.tile([ESP + P, HID], BF16, name=f"aggre{l}")
              for l in range(NLAY)]
    aggr_n = [[dram.tile([NQR + P, HID], BF16, name=f"aggrn{l}_{q}")
               for q in range(NQ)] for l in range(2 * NLAY)]
    ar_out = [dram.tile([NP_, HID], BF16, name=f"arout{l}")
              for l in range(2 * NLAY)]
    zeros_d = dram.tile([BLK, HID], BF16, name="zerod")

    with tc.tile_pool(name="zinit", bufs=1) as zp:
        zt = zp.tile([P, SUB * HID], BF16)
        nc.gpsimd.memset(zt[:], 0.0)
        nc.sync.dma_start(
            out=zeros_d[:].rearrange("(n p) d -> p n d", p=P), in_=zt[:])

    def zero_rows(tab, rows):
        r = 0
        while r < rows:
            n = min(BLK, rows - r)
            nc.sync.dma_start(out=tab[r:r + n, :], in_=zeros_d[0:n, :])
            r += n

    # ar_out pad rows [N, NP_) are never written by the AllReduce: zero them
    # once so the replicated node post-phase can't read NaNs into x pads.
    if NP_ > N:
        for l in range(2 * NLAY):
            nc.sync.dma_start(out=ar_out[l][N:NP_, :],
                              in_=zeros_d[0:NP_ - N, :])

    def t_rearr(ap):
        return ap.rearrange("(n p) d -> p n d", p=P)

    # ------------------------------------------------------------------
    # shared LN helper in transposed [HID, 512] layout (embeddings)
    # ------------------------------------------------------------------
    def ln_T(sp, pq, ps_in, bias_ap, lnci, tag):
        """ps_in: PSUM [HID,512] f32 = pre-LN linear output (no bias yet).
        Returns SBUF bf16 [HID,512] tile of silu(ln(x+b))."""
        xb = sp.tile([P, 512], BF16, name=f"{tag}_xb", tag=f"{tag}xb")
        nc.vector.tensor_scalar(xb[:HID, :], ps_in[:], bias_ap, None,
                                op0=OP.add)
        nc.vector.tensor_mul(xb[HID:, :], xb[:HID, :], xb[:HID, :])
        st = pq.tile([1, 512], F32, name=f"{tag}_st", tag=f"{tag}st")
        nc.tensor.matmul(st[:], lhsT=K.stS[:], rhs=xb[:], start=True,
                         stop=True)
        mean = sp.tile([1, 512], F32, name=f"{tag}_mean", tag=f"{tag}mn")
        nc.vector.tensor_scalar_mul(mean[:], st[:], 1.0 / HID)
        stq = pq.tile([1, 512], F32, name=f"{tag}_stq", tag=f"{tag}sq2")
        nc.tensor.matmul(stq[:], lhsT=K.stQ[:], rhs=xb[:], start=True,
                         stop=True)
        var = sp.tile([1, 512], F32, name=f"{tag}_var", tag=f"{tag}vr")
        nc.vector.tensor_scalar_mul(var[:], stq[:], 1.0 / HID)
        msq = sp.tile([1, 512], F32, name=f"{tag}_msq", tag=f"{tag}mq")
        nc.vector.tensor_mul(msq[:], mean[:], mean[:])
        nc.vector.tensor_sub(var[:], var[:], msq[:])
        sdv = sp.tile([1, 512], F32, name=f"{tag}_sdv", tag=f"{tag}sd")
        nc.scalar.activation(sdv[:], var[:], AF.Sqrt, bias=K.epsP[0:1, 0:1])
        rcp = sp.tile([1, 512], F32, name=f"{tag}_rcp", tag=f"{tag}rc")
        nc.vector.reciprocal(rcp[:], sdv[:])
        mrb = sp.tile([HID, 1024], F32, name=f"{tag}_mrb", tag=f"{tag}mb")
        nc.gpsimd.partition_broadcast(mrb[:, :512], mean[:])
        nc.gpsimd.partition_broadcast(mrb[:, 512:], rcp[:])
        t1 = sp.tile([HID, 512], F32, name=f"{tag}_t1", tag=f"{tag}t1")
        nc.vector.tensor_sub(t1[:], xb[:HID, :], mrb[:, :512])
        nc.vector.tensor_mul(t1[:], t1[:], mrb[:, 512:])
        nc.vector.tensor_scalar(t1[:], t1[:], K.lngc[:, lnci:lnci + 1],
                                K.lnbc[:, lnci:lnci + 1], op0=OP.mult,
                                op1=OP.add)
        sg_t = sp.tile([HID, 512], BF16, name=f"{tag}_sg", tag=f"{tag}sg")
        nc.scalar.activation(sg_t[:], t1[:], AF.Sigmoid)
        sl = sp.tile([HID, 512], BF16, name=f"{tag}_sl", tag=f"{tag}sl")
        nc.vector.tensor_mul(sl[:], t1[:], sg_t[:])
        return sl

    def rows_out(sp, pq, sl, out_tab, r0, tag):
        """Transpose [HID,512] bf16 back to rows and DMA to out_tab[r0:r0+512]."""
        tb = pq.tile([P, 256], BF16, name=f"{tag}_tb", tag=f"{tag}tb")
        for tt in range(4):
            nc.tensor.transpose(tb[:, tt * HID:(tt + 1) * HID],
                                sl[:, tt * P:(tt + 1) * P],
                                K.ident[:HID, :HID])
        ro = sp.tile([P, 256], BF16, name=f"{tag}_ro", tag=f"{tag}ro")
        nc.vector.tensor_copy(ro[:], tb[:])
        nc.sync.dma_start(out=t_rearr(out_tab[r0:r0 + 512, :])
                          if isinstance(r0, int)
                          else t_rearr(out_tab[ds(r0, 512), :]), in_=ro[:])

    # ------------------------------------------------------------------
    # embeddings
    # ------------------------------------------------------------------
    def emb_rbf(dist_in, L, cent, gam, Wt, bias_ap, lnci, out_rows, out_T,
                tag):
        nbins = cent.shape[0]
        nt = L // BLK
        with tc.tile_pool(name=f"{tag}_sb", bufs=3) as sp, \
             tc.tile_pool(name=f"{tag}_ps", bufs=2, space="PSUM") as pp, \
             tc.tile_pool(name=f"{tag}_pq", bufs=1, space="PSUM") as pq:
            with tc.For_i(0, nt * SUB, SUB) as it:
                dchunk = sp.tile([1, BLK], F32, name=f"{tag}_dch")
                nc.sync.dma_start(out=dchunk[:],
                                  in_=dist_in[None, ds(it * P, BLK)])
                for g in range(GRP):
                    gsl = slice(g * 512, (g + 1) * 512)
                    dbc = sp.tile([nbins, 512], F32, name=f"{tag}_dbc",
                                  tag="dbc")
                    nc.gpsimd.partition_broadcast(dbc[:], dchunk[:, gsl])
                    nc.vector.tensor_scalar(dbc[:], dbc[:], cent[:, 0:1],
                                            None, op0=OP.subtract)
                    sqv = sp.tile([nbins, 512], F32, name=f"{tag}_sqv",
                                  tag="sqv")
                    nc.vector.tensor_mul(sqv[:], dbc[:], dbc[:])
                    rbf = sp.tile([nbins, 512], BF16, name=f"{tag}_rbf",
                                  tag="rbf")
                    nc.scalar.activation(rbf[:], sqv[:], AF.Exp, scale=-gam)
                    ps = pp.tile([HID, 512], F32, name=f"{tag}_ps0")
                    nc.tensor.matmul(ps[:], lhsT=Wt[:], rhs=rbf[:],
                                     start=True, stop=True)
                    sl = ln_T(sp, pq, ps, bias_ap, lnci, tag)
                    if out_T is not None:
                        nc.sync.dma_start(
                            out=out_T[:, ds(it * P + g * 512, 512)],
                            in_=sl[:])
                    if out_rows is not None:
                        rows_out(sp, pq, sl, out_rows, it * P + g * 512, tag)

    def x_emb():
        nt = NP_ // BLK
        xa = ins["x_atom"]
        with tc.tile_pool(name="xe_sb", bufs=3) as sp, \
             tc.tile_pool(name="xe_ps", bufs=2, space="PSUM") as pp, \
             tc.tile_pool(name="xe_pq", bufs=1, space="PSUM") as pq:
            with tc.For_i(0, nt * SUB, SUB) as it:
                for g in range(GRP):
                    xt = sp.tile([P, 4 * ATOM], F32, name="xe_xt", tag="xt")
                    nc.sync.dma_start(
                        out=xt[:].rearrange("p (n d) -> p n d", d=ATOM),
                        in_=t_rearr(xa[ds(it * P + g * 512, 512), :]))
                    tp = pp.tile([ATOM, 512], F32, name="xe_tp")
                    for tt in range(4):
                        nc.tensor.transpose(tp[:, tt * P:(tt + 1) * P],
                                            xt[:, tt * ATOM:(tt + 1) * ATOM],
                                            K.identf[:])
                    tps = sp.tile([ATOM, 512], BF16, name="xe_tps", tag="tps")
                    nc.vector.tensor_copy(tps[:], tp[:])
                    ps = pp.tile([HID, 512], F32, name="xe_ps0")
                    nc.tensor.matmul(ps[:], lhsT=K.Wat[:], rhs=tps[:],
                                     start=True, stop=True)
                    sl = ln_T(sp, pq, ps, K.bemb[:, 0:1], NE + 0, "xe")
                    rows_out(sp, pq, sl, x_bufs[0], it * P + g * 512, "xe")

    # ------------------------------------------------------------------
    # AllToAll send gather
    # ------------------------------------------------------------------
    def send_a2a(l, e_src):
        nt = (C * PADM) // BLK
        with tc.tile_pool(name="snd_sb", bufs=3) as sp:
            with tc.For_i(0, nt * SUB, SUB) as it:
                six = sp.tile([P, SUB], I32, name="snd_six")
                nc.sync.dma_start(out=six[:],
                                  in_=ins["sendix"][:, ds(it, SUB)])
                gt = sp.tile([P, SUB * HID], BF16, name="snd_gt")
                for j in range(SUB):
                    nc.gpsimd.indirect_dma_start(
                        out=gt[:, j * HID:(j + 1) * HID], out_offset=None,
                        in_=e_src[:],
                        in_offset=IndirectOffsetOnAxis(ap=six[:, j:j + 1],
                                                       axis=0))
                nc.sync.dma_start(
                    out=t_rearr(send_b[l][ds(it * P, BLK), :]), in_=gt[:])
        nc.gpsimd.collective_compute(
            "AllToAll", OP.bypass, replica_groups=RG,
            ins=[send_b[l][:]], outs=[recv_b[l][:]])

    # ------------------------------------------------------------------
    # gate + scatter phase
    # ------------------------------------------------------------------
    def gate_phase(li, n_tiles, idx, dst_tab, src_tab, att_T, att_tab,
                   att_idx, aggr_tab, aggr_rows, tag):
        with tc.tile_pool(name=f"{tag}_sb", bufs=3) as sp, \
             tc.tile_pool(name=f"{tag}_p2", bufs=2, space="PSUM") as pp2, \
             tc.tile_pool(name=f"{tag}_p1", bufs=1, space="PSUM") as pp1:
            with tc.For_i(0, n_tiles, SUB) as it:
                gdx = sp.tile([P, SUB], I32, name=f"{tag}_gdx", tag="gdx")
                nc.sync.dma_start(out=gdx[:], in_=idx["gd"][:, ds(it, SUB)])
                gsx = sp.tile([P, SUB], I32, name=f"{tag}_gsx", tag="gsx")
                nc.sync.dma_start(out=gsx[:], in_=idx["gs"][:, ds(it, SUB)])
                scx = sp.tile([P, SUB], I32, name=f"{tag}_scx", tag="scx")
                nc.sync.dma_start(out=scx[:], in_=idx["sc"][:, ds(it, SUB)])
                sdc = sp.tile([P, SUB], F32, name=f"{tag}_sdc", tag="sdc")
                nc.sync.dma_start(out=sdc[:], in_=idx["sd"][:, ds(it, SUB)])
                sdt = sp.tile([1, BLK], F32, name=f"{tag}_sdt", tag="sdt")
                nc.sync.dma_start(out=sdt[:],
                                  in_=idx["sdT"][:, ds(it * P, BLK)])

                Gd = sp.tile([P, SUB * HID], BF16, name=f"{tag}_Gd", tag="Gd")
                Gs = sp.tile([P, SUB * HID], BF16, name=f"{tag}_Gs", tag="Gs")
                for j in range(SUB):
                    nc.gpsimd.indirect_dma_start(
                        out=Gd[:, j * HID:(j + 1) * HID], out_offset=None,
                        in_=dst_tab[:],
                        in_offset=IndirectOffsetOnAxis(ap=gdx[:, j:j + 1],
                                                       axis=0))
                    nc.gpsimd.indirect_dma_start(
                        out=Gs[:, j * HID:(j + 1) * HID], out_offset=None,
                        in_=src_tab[:],
                        in_offset=IndirectOffsetOnAxis(ap=gsx[:, j:j + 1],
                                                       axis=0))
                if att_T is None:
                    gax = sp.tile([P, SUB], I32, name=f"{tag}_gax", tag="gax")
                    nc.sync.dma_start(out=gax[:], in_=att_idx[:, ds(it, SUB)])
                    Ga = sp.tile([P, SUB * HID], BF16, name=f"{tag}_Ga",
                                 tag="Ga")
                    for j in range(SUB):
                        nc.gpsimd.indirect_dma_start(
                            out=Ga[:, j * HID:(j + 1) * HID], out_offset=None,
                            in_=att_tab[:],
                            in_offset=IndirectOffsetOnAxis(ap=gax[:, j:j + 1],
                                                           axis=0))

                Sc = sp.tile([P, SUB * HID], BF16, name=f"{tag}_Sc", tag="Sc")
                for g in range(GRP):
                    pk = pp2.tile([P, 512], BF16, name=f"{tag}_pk")
                    for tt in range(4):
                        j = g * 4 + tt
                        nc.tensor.transpose(
                            pk[:HID, tt * P:(tt + 1) * P],
                            Gd[:, j * HID:(j + 1) * HID], K.ident[:])
                        nc.tensor.transpose(
                            pk[HID:, tt * P:(tt + 1) * P],
                            Gs[:, j * HID:(j + 1) * HID], K.ident[:])
                    pks = sp.tile([P, 512], BF16, name=f"{tag}_pks", tag="pks")
                    nc.vector.tensor_copy(pks[:], pk[:])
                    if att_T is None:
                        pe = pp1.tile([HID, 512], BF16, name=f"{tag}_pe")
                        for tt in range(4):
                            j = g * 4 + tt
                            nc.tensor.transpose(
                                pe[:, tt * P:(tt + 1) * P],
                                Ga[:, j * HID:(j + 1) * HID], K.ident[:])
                        att_sb = sp.tile([HID, 512], BF16,
                                         name=f"{tag}_att", tag="att")
                        nc.vector.tensor_copy(att_sb[:], pe[:])
                    else:
                        att_sb = sp.tile([HID, 512], BF16,
                                         name=f"{tag}_attT", tag="att")
                        nc.sync.dma_start(
                            out=att_sb[:],
                            in_=att_T[:, ds(it * P + g * 512, 512)])

                    gu = pp2.tile([P, 512], F32, name=f"{tag}_gu")
                    nc.tensor.matmul(gu[:HID, :],
                                     lhsT=K.Wsd[:, li * HID:(li + 1) * HID],
                                     rhs=pks[:], start=True, stop=False)
                    nc.tensor.matmul(gu[:HID, :],
                                     lhsT=K.Weg[:, li * HID:(li + 1) * HID],
                                     rhs=att_sb[:], start=False, stop=True)
                    nc.tensor.matmul(gu[HID:, :],
                                     lhsT=K.Wdu[HID:, li * HID:(li + 1) * HID],
                                     rhs=pks[HID:, :], start=True, stop=True)
                    zt = sp.tile([HID, 512], BF16, name=f"{tag}_zt",
                                 tag="zt")
                    nc.vector.tensor_scalar(zt[:], gu[:HID, :],
                                            K.bgate[:, li:li + 1], None,
                                            op0=OP.add)
                    sgm = sp.tile([HID, 512], BF16, name=f"{tag}_sgm",
                                  tag="sgm")
                    nc.scalar.activation(sgm[:], zt[:], AF.Sigmoid)
                    gate = sp.tile([HID, 512], BF16, name=f"{tag}_gate",
                                   tag="gate")
                    nc.vector.tensor_mul(gate[:], zt[:], sgm[:])
                    ub = sp.tile([HID, 512], BF16, name=f"{tag}_ub", tag="ub")
                    nc.vector.tensor_scalar(ub[:], gu[HID:, :],
                                            K.bdu[:, li:li + 1], None,
                                            op0=OP.add)
                    msg = sp.tile([HID, 512], BF16, name=f"{tag}_msg",
                                  tag="msg")
                    nc.vector.tensor_mul(msg[:], gate[:], ub[:])
                    mt = pp1.tile([P, 256], BF16, name=f"{tag}_mt")
                    for tt in range(4):
                        nc.tensor.transpose(mt[:, tt * HID:(tt + 1) * HID],
                                            msg[:, tt * P:(tt + 1) * P],
                                            K.ident[:HID, :HID])
                    mts = sp.tile([P, 256], BF16, name=f"{tag}_mts", tag="mts")
                    nc.vector.tensor_copy(mts[:], mt[:])
                    sel = pp1.tile([P, 256], F32, name=f"{tag}_sel")
                    for tt in range(4):
                        j = g * 4 + tt
                        sb2 = pp1.tile([P, P], F32, name=f"{tag}_sb2")
                        nc.tensor.matmul(sb2[:], lhsT=K.ones1[:],
                                         rhs=sdt[:, j * P:(j + 1) * P],
                                         start=True, stop=True)
                        Sm = sp.tile([P, P], BF16, name=f"{tag}_Sm", tag="Sm")
                        nc.vector.tensor_tensor(
                            Sm[:], sdc[:, j:j + 1].to_broadcast([P, P]),
                            sb2[:], op=OP.is_equal)
                        nc.tensor.matmul(sel[:, tt * HID:(tt + 1) * HID],
                                         lhsT=Sm[:],
                                         rhs=mts[:, tt * HID:(tt + 1) * HID],
                                         start=True, stop=True)
                    nc.vector.tensor_copy(Sc[:, g * 256:(g + 1) * 256],
                                          sel[:])
                for j in range(SUB):
                    nc.gpsimd.indirect_dma_start(
                        out=aggr_tab[:],
                        out_offset=IndirectOffsetOnAxis(ap=scx[:, j:j + 1],
                                                        axis=0),
                        in_=Sc[:, j * HID:(j + 1) * HID], in_offset=None)

    # ------------------------------------------------------------------
    # post phase: silu(ln(su(src) + aggr)) + src -> out
    # ------------------------------------------------------------------
    def post_phase(li, rows, src_tab, aggr_tab, out_tab, tag):
        nt = rows // BLK
        with tc.tile_pool(name=f"{tag}_sb", bufs=3) as sp, \
             tc.tile_pool(name=f"{tag}_ps", bufs=2, space="PSUM") as pp:
            with tc.For_i(0, nt * SUB, SUB) as it:
                et = sp.tile([P, SUB * HID], BF16, name=f"{tag}_et", tag="et")
                nc.sync.dma_start(
                    out=et[:].rearrange("p (n d) -> p n d", d=HID),
                    in_=t_rearr(src_tab[ds(it * P, BLK), :]))
                ag = sp.tile([P, SUB * HID], BF16, name=f"{tag}_ag", tag="ag")
                nc.sync.dma_start(
                    out=ag[:].rearrange("p (n d) -> p n d", d=HID),
                    in_=t_rearr(aggr_tab[ds(it * P, BLK), :]))
                ob = sp.tile([P, SUB * HID], BF16, name=f"{tag}_ob", tag="ob")
                for g in range(GRP):
                    pe = pp.tile([HID, 512], BF16, name=f"{tag}_pe")
                    for tt in range(4):
                        j = g * 4 + tt
                        nc.tensor.transpose(pe[:, tt * P:(tt + 1) * P],
                                            et[:, j * HID:(j + 1) * HID],
                                            K.ident[:])
                    pes = sp.tile([HID, 512], BF16, name=f"{tag}_pes",
                                  tag="pes")
                    nc.vector.tensor_copy(pes[:], pe[:])
                    su = pp.tile([HID, 512], F32, name=f"{tag}_su")
                    nc.tensor.matmul(su[:],
                                     lhsT=K.Wsu[:, li * HID:(li + 1) * HID],
                                     rhs=pes[:], start=True, stop=True)
                    sus = sp.tile([HID, 512], BF16, name=f"{tag}_sus",
                                  tag="sus")
                    nc.vector.tensor_scalar(sus[:], su[:],
                                            K.bsu[:, li:li + 1], None,
                                            op0=OP.add)
                    sb_ = pp.tile([P, 256], BF16, name=f"{tag}_sb2")
                    for tt in range(4):
                        nc.tensor.transpose(sb_[:, tt * HID:(tt + 1) * HID],
                                            sus[:, tt * P:(tt + 1) * P],
                                            K.ident[:HID, :HID])
                    tsb = sp.tile([P, 256], F32, name=f"{tag}_tsb", tag="tsb")
                    nc.vector.tensor_add(tsb[:], sb_[:],
                                         ag[:, g * 256:(g + 1) * 256])
                    # LayerNorm over 64-feature groups, row-major
                    t3 = tsb[:].rearrange("p (n d) -> p n d", d=HID)
                    mean = sp.tile([P, 4], F32, name=f"{tag}_mean", tag="mn")
                    nc.vector.tensor_reduce(mean[:], t3, axis=AX.X, op=OP.add)
                    nc.vector.tensor_scalar_mul(mean[:], mean[:], 1.0 / HID)
                    cen = sp.tile([P, 256], F32, name=f"{tag}_cen", tag="cn")
                    nc.vector.tensor_tensor(
                        cen[:].rearrange("p (n d) -> p n d", d=HID), t3,
                        _bcast_mid(mean[:], 4, HID), op=OP.subtract)
                    c3 = cen[:].rearrange("p (n d) -> p n d", d=HID)
                    sqf = sp.tile([P, 256], F32, name=f"{tag}_sqf", tag="sq")
                    nc.vector.tensor_mul(
                        sqf[:].rearrange("p (n d) -> p n d", d=HID), c3, c3)
                    ssq = sp.tile([P, 4], F32, name=f"{tag}_ssq", tag="ssq")
                    nc.vector.tensor_reduce(
                        ssq[:], sqf[:].rearrange("p (n d) -> p n d", d=HID),
                        axis=AX.X, op=OP.add)
                    sdv = sp.tile([P, 4], F32, name=f"{tag}_sdv", tag="sdv")
                    nc.scalar.activation(sdv[:], ssq[:], AF.Sqrt,
                                         bias=K.epsP[:, 0:1],
                                         scale=1.0 / HID)
                    rcp = sp.tile([P, 4], F32, name=f"{tag}_rcp", tag="rcp")
                    nc.vector.reciprocal(rcp[:], sdv[:])
                    nc.vector.tensor_tensor(c3, c3, _bcast_mid(rcp[:], 4, HID),
                                            op=OP.mult)
                    nc.vector.tensor_tensor(c3, c3,
                                            _bcast_row(K.lng[li][:], 4, HID),
                                            op=OP.mult)
                    nc.vector.tensor_tensor(c3, c3,
                                            _bcast_row(K.lnb[li][:], 4, HID),
                                            op=OP.add)
                    sgm = sp.tile([P, 256], BF16, name=f"{tag}_psg",
                                  tag="psg")
                    nc.scalar.activation(sgm[:], cen[:], AF.Sigmoid)
                    slu = sp.tile([P, 256], F32, name=f"{tag}_slu", tag="sl")
                    nc.vector.tensor_mul(slu[:], cen[:], sgm[:])
                    nc.vector.tensor_add(ob[:, g * 256:(g + 1) * 256], slu[:],
                                         et[:, g * 256:(g + 1) * 256])
                nc.sync.dma_start(
                    out=t_rearr(out_tab[ds(it * P, BLK), :]),
                    in_=ob[:].rearrange("p (n d) -> p n d", d=HID))

    # ------------------------------------------------------------------
    # node EGC (gates + chunked AllReduce + replicated post)
    # ------------------------------------------------------------------
    def node_egc(li, lslot, x_in, x_out, e_tab, tag):
        aq = aggr_n[lslot]
        ar_o = ar_out[lslot]
        for q in range(NQ):
            zero_rows(aq[q], NQR)
        for q in range(NQ):
            nidx = {"gd": ins[f"n_gxd{q}"], "gs": ins[f"n_gxs{q}"],
                    "sc": ins[f"n_sc{q}"], "sd": ins[f"n_sd{q}"],
                    "sdT": ins[f"n_sdT{q}"]}
            gate_phase(li, LNQ[q] // P, nidx, x_in, x_in, None, e_tab,
                       ins[f"n_ge{q}"], aq[q], NQR, f"{tag}g{q}")
            nc.gpsimd.collective_compute(
                "AllReduce", OP.add, replica_groups=RG,
                ins=[aq[q][:NQR, :]],
                outs=[ar_o[q * NQR:(q + 1) * NQR, :]])
        post_phase(li, NP_, x_in, ar_o, x_out, f"{tag}p")

    # ------------------------------------------------------------------
    # readout
    # ------------------------------------------------------------------
    def readout(x_fin):
        nt = NP_ // BLK
        with tc.tile_pool(name="ro_sb", bufs=3) as sp, \
             tc.tile_pool(name="ro_acc", bufs=1) as ac, \
             tc.tile_pool(name="ro_ps", bufs=2, space="PSUM") as pp:
            accs = []
            for h in range(GH):
                a = ac.tile([P, HID + 1], F32, name=f"ro_acc{h}")
                nc.gpsimd.memset(a[:], 0.0)
                accs.append(a)
            with tc.For_i(0, nt * SUB, SUB) as it:
                xt = sp.tile([P, SUB * HID], BF16, name="ro_xt", tag="xt")
                nc.sync.dma_start(
                    out=xt[:].rearrange("p (n d) -> p n d", d=HID),
                    in_=t_rearr(x_fin[ds(it * P, BLK), :]))
                bt = sp.tile([P, SUB], F32, name="ro_bt", tag="bt")
                nc.sync.dma_start(out=bt[:], in_=ins["batchf"][:, ds(it, SUB)])
                for j in range(SUB):
                    xa = sp.tile([P, HID + 1], BF16, name="ro_xa", tag="xa")
                    nc.vector.tensor_copy(xa[:, :HID],
                                          xt[:, j * HID:(j + 1) * HID])
                    nc.gpsimd.memset(xa[:, HID:], 1.0)
                    M = sp.tile([P, GH * P], BF16, name="ro_M", tag="M")
                    nc.vector.tensor_tensor(
                        M[:], K.iotaf[:],
                        bt[:, j:j + 1].to_broadcast([P, GH * P]),
                        op=OP.is_equal)
                    for h in range(GH):
                        po = pp.tile([P, HID + 1], F32, name="ro_po")
                        nc.tensor.matmul(po[:], lhsT=M[:, h * P:(h + 1) * P],
                                         rhs=xa[:], start=True, stop=True)
                        nc.vector.tensor_add(accs[h][:], accs[h][:], po[:])
            out_sb = sp.tile([P, GH], F32, name="ro_out")
            for h in range(GH):
                cnt = sp.tile([P, 1], F32, name="ro_cnt", tag="cnt")
                nc.vector.tensor_scalar_max(cnt[:], accs[h][:, HID:], 1.0)
                rc = sp.tile([P, 1], F32, name="ro_rc", tag="rc")
                nc.vector.reciprocal(rc[:], cnt[:])
                pool = sp.tile([P, HID], BF16, name="ro_pool", tag="pool")
                nc.vector.tensor_tensor(pool[:], accs[h][:, :HID],
                                        rc[:].to_broadcast([P, HID]),
                                        op=OP.mult)
                pt = pp.tile([HID, P], BF16, name="ro_pt")
                nc.tensor.transpose(pt[:], pool[:], K.ident[:])
                pts = sp.tile([HID, P], BF16, name="ro_pts", tag="pts")
                nc.vector.tensor_copy(pts[:], pt[:])
                fc = pp.tile([HID, P], F32, name="ro_fc")
                nc.tensor.matmul(fc[:], lhsT=K.Wfc[:], rhs=pts[:],
                                 start=True, stop=True)
                zf = sp.tile([HID, P], F32, name="ro_zf", tag="zf")
                nc.vector.tensor_scalar(zf[:], fc[:], K.bfc[:, 0:1], None,
                                        op0=OP.add)
                sgf = sp.tile([HID, P], BF16, name="ro_sgf", tag="sgf")
                nc.scalar.activation(sgf[:], zf[:], AF.Sigmoid)
                hT = sp.tile([HID, P], BF16, name="ro_hT", tag="hT")
                nc.vector.tensor_mul(hT[:], zf[:], sgf[:])
                oo = pp.tile([P, 1], F32, name="ro_oo")
                nc.tensor.matmul(oo[:], lhsT=hT[:], rhs=K.Wout[:],
                                 start=True, stop=True)
                nc.scalar.activation(out_sb[:, h:h + 1], oo[:], AF.Identity,
                                     bias=K.boutP[:, 0:1])
            for h in range(GH):
                n = min(P, G - h * P)
                if n > 0:
                    nc.sync.dma_start(out=outs["out"][ds(h * P, n), None],
                                      in_=out_sb[:n, h:h + 1])

    # ------------------------------------------------------------------
    # program
    # ------------------------------------------------------------------
    x_emb()
    emb_rbf(ins["edist"], ESP, K.cent_e, meta["gam_e"], K.Wed,
            K.bemb[:, 1:2], NE + 1, e_bufs[0], None, "ee")
    emb_rbf(ins["angp"], LT, K.cent_a, meta["gam_a"], K.Wan,
            K.bemb[:, 2:3], NE + 2, None, aT, "ae")

    tidx = {k: ins["t_" + k] for k in ["gd", "gs", "sc", "sd", "sdT"]}
    send_a2a(0, e_bufs[0])
    for l in range(NLAY):
        zero_rows(aggr_e[l], ESP)
        gate_phase(2 * l, LT // P, tidx, e_bufs[l], recv_b[l], aT, None,
                   None, aggr_e[l], ES, f"eg{l}")
        post_phase(2 * l, ESP, e_bufs[l], aggr_e[l], e_bufs[l + 1],
                   f"ep{l}")
        if l + 1 < NLAY:
            send_a2a(l + 1, e_bufs[l + 1])
        node_egc(2 * l + 1, l, x_bufs[l], x_bufs[l + 1], e_bufs[l + 1],
                 f"na{l}")
    for gg in range(NLAY):
        node_egc(2 * NLAY + gg, NLAY + gg, x_bufs[NLAY + gg],
                 x_bufs[NLAY + gg + 1], e_bufs[NLAY], f"ng{gg}")

    readout(x_bufs[2 * NLAY])

    if meta.get("dbg"):
        for nm, tl in [("dbg_e0", e_bufs[0]), ("dbg_e1", e_bufs[1]),
                       ("dbg_x0", x_bufs[0]), ("dbg_x1", x_bufs[1]),
                       ("dbg_ag", aggr_e[0]), ("dbg_rv", recv_b[0]), ("dbg_sd", send_b[0]),
                       ("dbg_ar", ar_out[0]), ("dbg_xf", x_bufs[2 * NLAY])]:
            if nm in outs:
                nc.sync.dma_start(out=outs[nm], in_=tl[:])

    dram.release()
    K.pool.release()


# ----------------------------------------------------------------------------
# Runner
# ----------------------------------------------------------------------------

_DT = {np.dtype(np.float32): F32, np.dtype(BF): BF16,
       np.dtype(np.int32): I32}


def build_nc(meta, in_map0):
    C = meta["C"]
    nc = bacc.Bacc("TRN2", target_bir_lowering=False, debug=False,
                   num_devices=C)
    ins = {}
    for k, v in in_map0.items():
        t = nc.dram_tensor(k, list(v.shape), _DT[np.dtype(v.dtype)],
                           kind="ExternalInput")
        ins[k] = t[:]
    out_t = nc.dram_tensor("out", [meta["G"]], F32, kind="ExternalOutput")
    outs = {"out": out_t[:]}
    if meta.get("dbg"):
        ESP, NP_, NQR = meta["ESP"], meta["NP"], meta["NQR"]
        CP = meta["C"] * meta["PADM"]
        for nm, shp in [("dbg_e0", [ESP, HID]), ("dbg_e1", [ESP, HID]),
                        ("dbg_x0", [NP_, HID]), ("dbg_x1", [NP_, HID]),
                        ("dbg_ag", [ESP + P, HID]), ("dbg_rv", [CP, HID]), ("dbg_sd", [CP, HID]),
                        ("dbg_ar", [NP_, HID]), ("dbg_xf", [NP_, HID])]:
            outs[nm] = nc.dram_tensor(nm, shp, BF16, kind="ExternalOutput")[:]
    with tile.TileContext(nc) as tc:
        build(tc, outs, ins, meta)
    nc.compile()
    return nc


def kernel(**inputs):
    from concourse import bass_utils
    meta, in_maps = prep(inputs, C=8, BLK=4096)
    nc = build_nc(meta, in_maps[0])
    res = bass_utils.run_bass_kernel_spmd(nc, in_maps,
                                          core_ids=list(range(meta["C"])))
    return np.asarray(res.results[0]["out"], dtype=np.float32)


if __name__ == "__main__":
    pass


# revision 23
# speedup vs baseline: 1.1432x; 1.1432x over previous
"""ALIGNN (nn_ALIGNN_PyG) distributed Trainium2 Bass kernel, 8 NeuronCores.

Sharding (graph-data parallel, comm-minimized):
  - e-rows (line-graph nodes, E) sharded contiguously: E/8 rows per core.
  - Triplets assigned to the owner of their dst edge, processed in dst-sorted
    order, packed into 128-row tiles aligned to segment boundaries (host-side
    padding) so per-tile selection-matmul segment sums never straddle a tile
    and each output row is written by exactly one scatter descriptor.
  - Edge gates need e[src] rows owned by other cores -> AllToAll of the
    unique requested rows (host-computed routing tables).
  - Node-graph edges processed at the core owning the e-row; per-core partial
    aggregates over all N nodes are AllReduced in 4 row-chunks; the node
    post-phase is computed replicated so x stays replicated on all cores.
  - bf16 storage/streams, fp32 PSUM/LN statistics.

All index manipulation (sorting, routing, padding) happens on the host in
numpy; all floating-point math runs on device.
"""

import sys

sys.path.insert(0, "/opt/trn_rl_repo")

import numpy as np
import ml_dtypes

from concourse import bass, bacc, mybir, tile
from concourse.bass import ds, IndirectOffsetOnAxis
from concourse.masks import make_identity

F32 = mybir.dt.float32
BF16 = mybir.dt.bfloat16
I32 = mybir.dt.int32

P = 128
HID = 64
ATOM = 92
EBINS = 40
TBINS = 20
RADIUS = 10.0
NLAY = 4
OOB = 1 << 30
PADSEG = 300.0
EPS = 1e-5
AX = mybir.AxisListType
AF = mybir.ActivationFunctionType
OP = mybir.AluOpType

BF = ml_dtypes.bfloat16


def _bf(x):
    return np.ascontiguousarray(np.asarray(x, dtype=np.float32)).astype(BF)


def _f32(x):
    return np.ascontiguousarray(np.asarray(x, dtype=np.float32))


def _rup(x, m):
    return ((x + m - 1) // m) * m


# ----------------------------------------------------------------------------
# Host preprocessing
# ----------------------------------------------------------------------------

def _segment_slots(d, PP=P):
    """d: sorted int array. Greedy-pack runs of equal values into PP-row tiles
    so no run straddles a tile boundary. Returns slots, per-element tile-local
    run labels (0..PP-1, the run's first slot within its tile; small ints so
    the PE-matmul broadcast of labels is exact even at reduced precision),
    and the tile count."""
    n = len(d)
    if n == 0:
        return np.zeros(0, np.int64), np.zeros(0, np.int64), 0
    bnd = np.flatnonzero(np.diff(d)) + 1
    starts = np.concatenate([[0], bnd]).tolist()
    ends = np.concatenate([bnd, [n]]).tolist()
    slot = np.empty(n, np.int64)
    lab = np.empty(n, np.int64)
    pos = 0
    ar = np.arange(PP)
    for s, e in zip(starts, ends):
        ln = e - s
        assert ln <= PP, f"segment run {ln} > {PP}"
        if (pos % PP) + ln > PP:
            pos = ((pos // PP) + 1) * PP
        slot[s:e] = ar[:ln] + pos
        lab[s:e] = pos % PP
        pos += ln
    return slot, lab, (pos + PP - 1) // PP


def _col128(x):
    """[L] stream -> [128, L//128] with column j = tile j."""
    return np.ascontiguousarray(x.reshape(-1, P).T)


def _pack_stream(dl, extras, dummy):
    """dl: sorted local dst ids. extras: {name: (aligned_array, pad_value)}.
    Masked (pad / non-first-of-segment) rows scatter to the dummy row."""
    slot, lab, nt = _segment_slots(dl)
    L = nt * P
    out = {}
    gd = np.zeros(L, np.int64)
    gd[slot] = dl
    sd = np.full(L, PADSEG, np.float32)
    sd[slot] = lab.astype(np.float32)
    sc = np.full(L, dummy, np.int64)
    if len(dl):
        first = np.ones(len(dl), bool)
        first[1:] = dl[1:] != dl[:-1]
        sc[slot[first]] = dl[first]
    out["gd"], out["sd"], out["sc"] = gd, sd, sc
    for k, (arr, padv) in extras.items():
        a = np.full(L, padv, arr.dtype)
        a[slot] = arr
        out[k] = a
    out["n"] = L
    return out


def _pad_to(st, L, pads):
    for k, padv in pads.items():
        a = st[k]
        if len(a) < L:
            st[k] = np.concatenate([a, np.full(L - len(a), padv, a.dtype)])
    st["n"] = L


_BASE_PADS = {"gd": np.int64(0), "sd": np.float32(PADSEG)}


def prep(inputs, C=8, BLK=4096):
    x_atom = _f32(inputs["x_atom"])
    edge_dist = _f32(inputs["edge_dist"])
    angle_cos = _f32(inputs["angle_cos"])
    params = inputs["params"]
    ei = np.asarray(inputs["edge_index"]).astype(np.int64)
    lg = np.asarray(inputs["lg_edge_index"]).astype(np.int64)
    batch = np.asarray(inputs["batch"]).astype(np.int64)
    G = int(np.asarray(inputs["num_graphs"]))

    N, E = x_atom.shape[0], edge_dist.shape[0]
    assert E % C == 0
    ES = E // C
    NP_ = _rup(N, BLK)
    ESP = _rup(ES, BLK)
    NQ = 4
    assert N % NQ == 0
    NQR = N // NQ

    meta = dict(C=C, N=N, E=E, G=G, ES=ES, NP=NP_, ESP=ESP, NQ=NQ, NQR=NQR,
                BLK=BLK)

    # ---- triplet (edge-EGC) streams ----------------------------------------
    src_t, dst_t = lg[0], lg[1]
    own_t = dst_t // ES
    trip = []
    for c in range(C):
        m = np.flatnonzero(own_t == c)
        o = m[np.argsort(dst_t[m], kind="stable")]
        dl = dst_t[o] - c * ES
        trip.append(_pack_stream(dl, {"sg": (src_t[o], np.int64(-1)),
                                      "ang": (angle_cos[o], np.float32(0))},
                                 ESP))
    LT = _rup(max(t["n"] for t in trip), BLK)
    for t in trip:
        _pad_to(t, LT, dict(_BASE_PADS, sc=np.int64(ESP), sg=np.int64(-1),
                            ang=np.float32(0)))
    meta["LT"] = LT

    # ---- AllToAll routing for e[src] rows ----------------------------------
    uniq = [[None] * C for _ in range(C)]
    for c in range(C):
        sgl = trip[c]["sg"]
        for s in range(C):
            sel = sgl[(sgl >= s * ES) & (sgl < (s + 1) * ES)]
            uniq[s][c] = np.unique(sel)
    PADM = max(max(len(uniq[s][c]) for c in range(C)) for s in range(C))
    PADM = max(PADM, 1)
    PADM = _rup(PADM, BLK // C) if (BLK % C == 0) else _rup(PADM, P)
    while (C * PADM) % BLK != 0:
        PADM += P
    meta["PADM"] = PADM
    send_idx = np.zeros((C, C * PADM), np.int64)
    for s in range(C):
        for c in range(C):
            ids = uniq[s][c] - s * ES
            send_idx[s, c * PADM:c * PADM + len(ids)] = ids
    for c in range(C):
        sgl = trip[c]["sg"]
        gs = np.zeros(LT, np.int64)
        for s in range(C):
            msk = (sgl >= s * ES) & (sgl < (s + 1) * ES)
            gs[msk] = s * PADM + np.searchsorted(uniq[s][c], sgl[msk])
        gs[sgl < 0] = 0
        trip[c]["gs"] = gs

    # ---- node-EGC streams, quartered for chunked AllReduce -----------------
    src_n, dst_n = ei[0], ei[1]
    node = []
    for c in range(C):
        j0, j1 = c * ES, (c + 1) * ES
        dd = dst_n[j0:j1]
        ss = src_n[j0:j1]
        qs = []
        for q in range(NQ):
            m = np.flatnonzero((dd >= q * NQR) & (dd < (q + 1) * NQR))
            o = m[np.argsort(dd[m], kind="stable")]
            dl = dd[o] - q * NQR
            qs.append(_pack_stream(dl, {"ge": (o.astype(np.int64), np.int64(0)),
                                        "gxs": (ss[o], np.int64(0)),
                                        "gxd": (dd[o], np.int64(0))}, NQR))
        node.append(qs)
    LNQ = [_rup(max(max(node[c][q]["n"] for c in range(C)), BLK), BLK)
           for q in range(NQ)]
    for c in range(C):
        for q in range(NQ):
            _pad_to(node[c][q], LNQ[q],
                    dict(_BASE_PADS, sc=np.int64(NQR), ge=np.int64(0),
                         gxs=np.int64(0), gxd=np.int64(0)))
    meta["LNQ"] = LNQ

    # ---- weights -----------------------------------------------------------
    def lin(p):
        return _f32(p["w"]), _f32(p["b"])

    egcs = []
    for l in range(NLAY):
        egcs.append(params["alignn"][l]["edge"])
        egcs.append(params["alignn"][l]["node"])
    egcs.extend(params["gcn"])
    NE = len(egcs)
    meta["NE"] = NE

    Wsd = np.zeros((P, NE * HID), np.float32)
    Weg = np.zeros((HID, NE * HID), np.float32)
    Wdu = np.zeros((P, NE * HID), np.float32)
    Wsu = np.zeros((HID, NE * HID), np.float32)
    bgate = np.zeros((HID, NE), np.float32)
    bdu = np.zeros((HID, NE), np.float32)
    bsu = np.zeros((HID, NE), np.float32)
    lngr = np.zeros((NE + 3, HID), np.float32)
    lnbr = np.zeros((NE + 3, HID), np.float32)
    for i, p in enumerate(egcs):
        sw, sb_ = lin(p["sg"])
        dw, db = lin(p["dg"])
        ew, eb = lin(p["eg"])
        uw, ub = lin(p["du"])
        tw, tb = lin(p["su"])
        Wsd[:HID, i * HID:(i + 1) * HID] = sw
        Wsd[HID:, i * HID:(i + 1) * HID] = dw
        Weg[:, i * HID:(i + 1) * HID] = ew
        Wdu[HID:, i * HID:(i + 1) * HID] = uw
        Wsu[:, i * HID:(i + 1) * HID] = tw
        bgate[:, i] = sb_ + db + eb
        bdu[:, i] = ub
        bsu[:, i] = tb
        lngr[i] = _f32(p["ln_g"])
        lnbr[i] = _f32(p["ln_b"])

    embs = [params["atom_emb"], params["edge_emb"], params["angle_emb"]]
    Wat = np.zeros((ATOM, HID), np.float32)
    Wed = np.zeros((EBINS, HID), np.float32)
    Wan = np.zeros((TBINS, HID), np.float32)
    bemb = np.zeros((HID, 3), np.float32)
    for i, p in enumerate(embs):
        w, b = lin(p)
        [Wat, Wed, Wan][i][:, :] = w
        bemb[:, i] = b
        lngr[NE + i] = _f32(p["ln_g"])
        lnbr[NE + i] = _f32(p["ln_b"])

    Wfc, bfc = lin(params["fc"])
    Wout, bout = lin(params["out"])

    cent_e = np.linspace(0.0, RADIUS, EBINS).astype(np.float32)
    gam_e = 1.0 / (cent_e[1] - cent_e[0]) ** 2
    cent_a = np.linspace(-1.0, 1.0, TBINS).astype(np.float32)
    gam_a = 1.0 / (cent_a[1] - cent_a[0]) ** 2
    meta["gam_e"], meta["gam_a"] = float(gam_e), float(gam_a)
    meta["bout"] = float(bout[0])
    meta["GH"] = _rup(G, P) // P

    xa_pad = np.zeros((NP_, ATOM), np.float32)
    xa_pad[:N] = x_atom
    bat_pad = np.full(NP_, 2.0e6, np.float32)
    bat_pad[:N] = batch.astype(np.float32)

    shared = dict(
        x_atom=xa_pad,
        batchf=_col128(bat_pad),
        Wsd=_bf(Wsd), Weg=_bf(Weg), Wdu=_bf(Wdu), Wsu=_bf(Wsu),
        bgate=bgate, bdu=bdu, bsu=bsu,
        lngr=lngr, lnbr=lnbr,
        lngc=np.ascontiguousarray(lngr.T), lnbc=np.ascontiguousarray(lnbr.T),
        Wat=_bf(Wat), Wed=_bf(Wed), Wan=_bf(Wan), bemb=bemb,
        Wfc=_bf(Wfc), bfc=_f32(bfc).reshape(HID, 1),
        Wout=_bf(Wout).reshape(HID, 1),
        cent_e=cent_e.reshape(EBINS, 1), cent_a=cent_a.reshape(TBINS, 1),
    )

    in_maps = []
    for c in range(C):
        ed_pad = np.zeros(ESP, np.float32)
        ed_pad[:ES] = edge_dist[c * ES:(c + 1) * ES]
        t = trip[c]
        m = dict(shared)
        m["edist"] = ed_pad
        m["angp"] = t["ang"]
        m["t_gd"] = _col128(t["gd"].astype(np.int32))
        m["t_gs"] = _col128(t["gs"].astype(np.int32))
        m["t_sc"] = _col128(t["sc"].astype(np.int32))
        m["t_sd"] = _col128(t["sd"])
        m["t_sdT"] = t["sd"].reshape(1, -1)
        m["sendix"] = _col128(send_idx[c].astype(np.int32))
        for q in range(NQ):
            nq = node[c][q]
            m[f"n_ge{q}"] = _col128(nq["ge"].astype(np.int32))
            m[f"n_gxs{q}"] = _col128(nq["gxs"].astype(np.int32))
            m[f"n_gxd{q}"] = _col128(nq["gxd"].astype(np.int32))
            m[f"n_sc{q}"] = _col128(nq["sc"].astype(np.int32))
            m[f"n_sd{q}"] = _col128(nq["sd"])
            m[f"n_sdT{q}"] = nq["sd"].reshape(1, -1)
        in_maps.append(m)

    return meta, in_maps


# ----------------------------------------------------------------------------
# Device kernel
# ----------------------------------------------------------------------------

def _bcast_mid(ap2d, nsub, inner):
    """[128, k] AP -> [128, (1,k)... wait: build [p, nsub, inner] view with the
    given free pattern pairs."""
    return bass.AP(ap2d.tensor, ap2d.offset, [ap2d.ap[0], (1, nsub),
                                              (0, inner)])


def _bcast_row(ap2d, nsub, inner):
    """[128, inner] AP -> [p, nsub(bcast), inner]."""
    return bass.AP(ap2d.tensor, ap2d.offset, [ap2d.ap[0], (0, nsub),
                                              (1, inner)])


class Consts:
    pass


def _load_consts(tc, nc, ins, meta):
    K = Consts()
    cp = tc.alloc_tile_pool(name="consts", bufs=1)
    K.pool = cp

    def sb(name):
        a = ins[name]
        t = cp.tile(list(a.shape), a.dtype, name="c_" + name)
        nc.sync.dma_start(out=t[:], in_=a[:])
        return t

    for nm in ["Wsd", "Weg", "Wdu", "Wsu", "bgate", "bdu", "bsu",
               "Wat", "Wed", "Wan", "bemb", "Wfc", "bfc", "Wout",
               "cent_e", "cent_a", "lngc", "lnbc"]:
        setattr(K, nm, sb(nm))

    NE = meta["NE"]
    K.lng = []
    K.lnb = []
    for i in range(NE):
        gr = cp.tile([1, HID], F32, name=f"lngr{i}")
        nc.sync.dma_start(out=gr[:], in_=ins["lngr"][i:i + 1, :])
        br = cp.tile([1, HID], F32, name=f"lnbr{i}")
        nc.sync.dma_start(out=br[:], in_=ins["lnbr"][i:i + 1, :])
        g = cp.tile([P, HID], F32, name=f"lng{i}")
        b = cp.tile([P, HID], F32, name=f"lnb{i}")
        nc.gpsimd.partition_broadcast(g[:], gr[:])
        nc.gpsimd.partition_broadcast(b[:], br[:])
        K.lng.append(g)
        K.lnb.append(b)

    K.ident = cp.tile([P, P], BF16, name="identbf")
    make_identity(nc, K.ident[:])
    K.identf = cp.tile([P, P], F32, name="identf")
    make_identity(nc, K.identf[:])

    K.ones1 = cp.tile([1, P], F32, name="ones1")
    nc.gpsimd.memset(K.ones1[:], 1.0)

    K.stS = cp.tile([P, 1], BF16, name="stS")
    nc.gpsimd.memset(K.stS[:], 0.0)
    nc.gpsimd.memset(K.stS[:HID, 0:1], 1.0)
    K.stQ = cp.tile([P, 1], BF16, name="stQ")
    nc.gpsimd.memset(K.stQ[:], 0.0)
    nc.gpsimd.memset(K.stQ[HID:, 0:1], 1.0)

    K.epsP = cp.tile([P, 1], F32, name="epsP")
    nc.gpsimd.memset(K.epsP[:], EPS)
    K.boutP = cp.tile([P, 1], F32, name="boutP")
    nc.gpsimd.memset(K.boutP[:], float(meta["bout"]))

    GH = meta["GH"]
    it = cp.tile([P, GH * P], I32, name="iotai")
    nc.gpsimd.iota(it[:], pattern=[[1, GH * P]], base=0, channel_multiplier=0)
    K.iotaf = cp.tile([P, GH * P], F32, name="iotaf")
    nc.vector.tensor_copy(K.iotaf[:], it[:])
    return K


def build(tc, outs, ins, meta):
    nc = tc.nc
    C, BLK = meta["C"], meta["BLK"]
    ES, ESP, NP_, LT = meta["ES"], meta["ESP"], meta["NP"], meta["LT"]
    NQ, NQR, LNQ = meta["NQ"], meta["NQR"], meta["LNQ"]
    PADM, NE, G, GH = meta["PADM"], meta["NE"], meta["G"], meta["GH"]
    N = meta["N"]
    SUB = BLK // P
    GRP = BLK // 512
    RG = [list(range(C))]

    K = _load_consts(tc, nc, ins, meta)

    dram = tc.alloc_tile_pool(name="dram", bufs=1, space="DRAM")
    e_bufs = [dram.tile([ESP, HID], BF16, name=f"e{l}")
              for l in range(NLAY + 1)]
    x_bufs = [dram.tile([NP_, HID], BF16, name=f"x{l}")
              for l in range(2 * NLAY + 1)]
    aT = dram.tile([HID, LT], BF16, name="aT")
    send_b = [dram.tile([C * PADM, HID], BF16, name=f"send{l}")
              for l in range(NLAY)]
    recv_b = [dram.tile([C * PADM, HID], BF16, name=f"recv{l}")
              for l in range(NLAY)]
    aggr_e = [dram.tile([ESP + P, HID], BF16, name=f"aggre{l}")
              for l in range(NLAY)]
    aggr_n = [[dram.tile([NQR + P, HID], BF16, name=f"aggrn{l}_{q}")
               for q in range(NQ)] for l in range(2 * NLAY)]
    ar_out = [dram.tile([NP_, HID], BF16, name=f"arout{l}")
              for l in range(2 * NLAY)]
    zeros_d = dram.tile([BLK, HID], BF16, name="zerod")

    with tc.tile_pool(name="zinit", bufs=1) as zp:
        zt = zp.tile([P, SUB * HID], BF16)
        nc.gpsimd.memset(zt[:], 0.0)
        nc.sync.dma_start(
            out=zeros_d[:].rearrange("(n p) d -> p n d", p=P), in_=zt[:])

    def zero_rows(tab, rows):
        r = 0
        while r < rows:
            n = min(BLK, rows - r)
            nc.sync.dma_start(out=tab[r:r + n, :], in_=zeros_d[0:n, :])
            r += n

    # ar_out pad rows [N, NP_) are never written by the AllReduce: zero them
    # once so the replicated node post-phase can't read NaNs into x pads.
    if NP_ > N:
        for l in range(2 * NLAY):
            nc.sync.dma_start(out=ar_out[l][N:NP_, :],
                              in_=zeros_d[0:NP_ - N, :])

    def t_rearr(ap):
        return ap.rearrange("(n p) d -> p n d", p=P)

    # ------------------------------------------------------------------
    # shared LN helper in transposed [HID, 512] layout (embeddings)
    # ------------------------------------------------------------------
    def ln_T(sp, pq, ps_in, bias_ap, lnci, tag):
        """ps_in: PSUM [HID,512] f32 = pre-LN linear output (no bias yet).
        Returns SBUF bf16 [HID,512] tile of silu(ln(x+b))."""
        xb = sp.tile([P, 512], BF16, name=f"{tag}_xb", tag=f"{tag}xb")
        nc.vector.tensor_scalar(xb[:HID, :], ps_in[:], bias_ap, None,
                                op0=OP.add)
        nc.vector.tensor_mul(xb[HID:, :], xb[:HID, :], xb[:HID, :])
        st = pq.tile([1, 512], F32, name=f"{tag}_st", tag=f"{tag}st")
        nc.tensor.matmul(st[:], lhsT=K.stS[:], rhs=xb[:], start=True,
                         stop=True)
        mean = sp.tile([1, 512], F32, name=f"{tag}_mean", tag=f"{tag}mn")
        nc.vector.tensor_scalar_mul(mean[:], st[:], 1.0 / HID)
        stq = pq.tile([1, 512], F32, name=f"{tag}_stq", tag=f"{tag}sq2")
        nc.tensor.matmul(stq[:], lhsT=K.stQ[:], rhs=xb[:], start=True,
                         stop=True)
        var = sp.tile([1, 512], F32, name=f"{tag}_var", tag=f"{tag}vr")
        nc.vector.tensor_scalar_mul(var[:], stq[:], 1.0 / HID)
        msq = sp.tile([1, 512], F32, name=f"{tag}_msq", tag=f"{tag}mq")
        nc.vector.tensor_mul(msq[:], mean[:], mean[:])
        nc.vector.tensor_sub(var[:], var[:], msq[:])
        sdv = sp.tile([1, 512], F32, name=f"{tag}_sdv", tag=f"{tag}sd")
        nc.scalar.activation(sdv[:], var[:], AF.Sqrt, bias=K.epsP[0:1, 0:1])
        rcp = sp.tile([1, 512], F32, name=f"{tag}_rcp", tag=f"{tag}rc")
        nc.vector.reciprocal(rcp[:], sdv[:])
        mrb = sp.tile([HID, 1024], F32, name=f"{tag}_mrb", tag=f"{tag}mb")
        nc.gpsimd.partition_broadcast(mrb[:, :512], mean[:])
        nc.gpsimd.partition_broadcast(mrb[:, 512:], rcp[:])
        t1 = sp.tile([HID, 512], F32, name=f"{tag}_t1", tag=f"{tag}t1")
        nc.vector.tensor_sub(t1[:], xb[:HID, :], mrb[:, :512])
        nc.vector.tensor_mul(t1[:], t1[:], mrb[:, 512:])
        nc.vector.tensor_scalar(t1[:], t1[:], K.lngc[:, lnci:lnci + 1],
                                K.lnbc[:, lnci:lnci + 1], op0=OP.mult,
                                op1=OP.add)
        sg_t = sp.tile([HID, 512], BF16, name=f"{tag}_sg", tag=f"{tag}sg")
        nc.scalar.activation(sg_t[:], t1[:], AF.Sigmoid)
        sl = sp.tile([HID, 512], BF16, name=f"{tag}_sl", tag=f"{tag}sl")
        nc.vector.tensor_mul(sl[:], t1[:], sg_t[:])
        return sl

    def rows_out(sp, pq, sl, out_tab, r0, tag):
        """Transpose [HID,512] bf16 back to rows and DMA to out_tab[r0:r0+512]."""
        tb = pq.tile([P, 256], BF16, name=f"{tag}_tb", tag=f"{tag}tb")
        for tt in range(4):
            nc.tensor.transpose(tb[:, tt * HID:(tt + 1) * HID],
                                sl[:, tt * P:(tt + 1) * P],
                                K.ident[:HID, :HID])
        ro = sp.tile([P, 256], BF16, name=f"{tag}_ro", tag=f"{tag}ro")
        nc.vector.tensor_copy(ro[:], tb[:])
        nc.sync.dma_start(out=t_rearr(out_tab[r0:r0 + 512, :])
                          if isinstance(r0, int)
                          else t_rearr(out_tab[ds(r0, 512), :]), in_=ro[:])

    # ------------------------------------------------------------------
    # embeddings
    # ------------------------------------------------------------------
    def emb_rbf(dist_in, L, cent, gam, Wt, bias_ap, lnci, out_rows, out_T,
                tag):
        nbins = cent.shape[0]
        nt = L // BLK
        with tc.tile_pool(name=f"{tag}_sb", bufs=3) as sp, \
             tc.tile_pool(name=f"{tag}_ps", bufs=2, space="PSUM") as pp, \
             tc.tile_pool(name=f"{tag}_pq", bufs=1, space="PSUM") as pq:
            with tc.For_i(0, nt * SUB, SUB, staggered_reset=True) as it:
                dchunk = sp.tile([1, BLK], F32, name=f"{tag}_dch")
                nc.sync.dma_start(out=dchunk[:],
                                  in_=dist_in[None, ds(it * P, BLK)])
                for g in range(GRP):
                    gsl = slice(g * 512, (g + 1) * 512)
                    dbc = sp.tile([nbins, 512], F32, name=f"{tag}_dbc",
                                  tag="dbc")
                    nc.gpsimd.partition_broadcast(dbc[:], dchunk[:, gsl])
                    nc.vector.tensor_scalar(dbc[:], dbc[:], cent[:, 0:1],
                                            None, op0=OP.subtract)
                    sqv = sp.tile([nbins, 512], F32, name=f"{tag}_sqv",
                                  tag="sqv")
                    nc.vector.tensor_mul(sqv[:], dbc[:], dbc[:])
                    rbf = sp.tile([nbins, 512], BF16, name=f"{tag}_rbf",
                                  tag="rbf")
                    nc.scalar.activation(rbf[:], sqv[:], AF.Exp, scale=-gam)
                    ps = pp.tile([HID, 512], F32, name=f"{tag}_ps0")
                    nc.tensor.matmul(ps[:], lhsT=Wt[:], rhs=rbf[:],
                                     start=True, stop=True)
                    sl = ln_T(sp, pq, ps, bias_ap, lnci, tag)
                    if out_T is not None:
                        nc.sync.dma_start(
                            out=out_T[:, ds(it * P + g * 512, 512)],
                            in_=sl[:])
                    if out_rows is not None:
                        rows_out(sp, pq, sl, out_rows, it * P + g * 512, tag)

    def x_emb():
        nt = NP_ // BLK
        xa = ins["x_atom"]
        with tc.tile_pool(name="xe_sb", bufs=3) as sp, \
             tc.tile_pool(name="xe_ps", bufs=2, space="PSUM") as pp, \
             tc.tile_pool(name="xe_pq", bufs=1, space="PSUM") as pq:
            with tc.For_i(0, nt * SUB, SUB, staggered_reset=True) as it:
                for g in range(GRP):
                    xt = sp.tile([P, 4 * ATOM], F32, name="xe_xt", tag="xt")
                    nc.sync.dma_start(
                        out=xt[:].rearrange("p (n d) -> p n d", d=ATOM),
                        in_=t_rearr(xa[ds(it * P + g * 512, 512), :]))
                    tp = pp.tile([ATOM, 512], F32, name="xe_tp")
                    for tt in range(4):
                        nc.tensor.transpose(tp[:, tt * P:(tt + 1) * P],
                                            xt[:, tt * ATOM:(tt + 1) * ATOM],
                                            K.identf[:])
                    tps = sp.tile([ATOM, 512], BF16, name="xe_tps", tag="tps")
                    nc.vector.tensor_copy(tps[:], tp[:])
                    ps = pp.tile([HID, 512], F32, name="xe_ps0")
                    nc.tensor.matmul(ps[:], lhsT=K.Wat[:], rhs=tps[:],
                                     start=True, stop=True)
                    sl = ln_T(sp, pq, ps, K.bemb[:, 0:1], NE + 0, "xe")
                    rows_out(sp, pq, sl, x_bufs[0], it * P + g * 512, "xe")

    # ------------------------------------------------------------------
    # AllToAll send gather
    # ------------------------------------------------------------------
    def send_a2a(l, e_src):
        nt = (C * PADM) // BLK
        with tc.tile_pool(name="snd_sb", bufs=3) as sp:
            with tc.For_i(0, nt * SUB, SUB, staggered_reset=True) as it:
                six = sp.tile([P, SUB], I32, name="snd_six")
                nc.sync.dma_start(out=six[:],
                                  in_=ins["sendix"][:, ds(it, SUB)])
                gt = sp.tile([P, SUB * HID], BF16, name="snd_gt")
                for j in range(SUB):
                    nc.gpsimd.indirect_dma_start(
                        out=gt[:, j * HID:(j + 1) * HID], out_offset=None,
                        in_=e_src[:],
                        in_offset=IndirectOffsetOnAxis(ap=six[:, j:j + 1],
                                                       axis=0))
                nc.sync.dma_start(
                    out=t_rearr(send_b[l][ds(it * P, BLK), :]), in_=gt[:])
        nc.gpsimd.collective_compute(
            "AllToAll", OP.bypass, replica_groups=RG,
            ins=[send_b[l][:]], outs=[recv_b[l][:]])

    # ------------------------------------------------------------------
    # gate + scatter phase
    # ------------------------------------------------------------------
    def gate_phase(li, n_tiles, idx, dst_tab, src_tab, att_T, att_tab,
                   att_idx, aggr_tab, aggr_rows, tag):
        with tc.tile_pool(name=f"{tag}_sb", bufs=3) as sp, \
             tc.tile_pool(name=f"{tag}_p2", bufs=2, space="PSUM") as pp2, \
             tc.tile_pool(name=f"{tag}_p1", bufs=1, space="PSUM") as pp1:
            with tc.For_i(0, n_tiles, SUB, staggered_reset=True) as it:
                gdx = sp.tile([P, SUB], I32, name=f"{tag}_gdx", tag="gdx")
                nc.sync.dma_start(out=gdx[:], in_=idx["gd"][:, ds(it, SUB)])
                gsx = sp.tile([P, SUB], I32, name=f"{tag}_gsx", tag="gsx")
                nc.sync.dma_start(out=gsx[:], in_=idx["gs"][:, ds(it, SUB)])
                scx = sp.tile([P, SUB], I32, name=f"{tag}_scx", tag="scx")
                nc.sync.dma_start(out=scx[:], in_=idx["sc"][:, ds(it, SUB)])
                sdc = sp.tile([P, SUB], F32, name=f"{tag}_sdc", tag="sdc")
                nc.sync.dma_start(out=sdc[:], in_=idx["sd"][:, ds(it, SUB)])
                sdt = sp.tile([1, BLK], F32, name=f"{tag}_sdt", tag="sdt")
                nc.sync.dma_start(out=sdt[:],
                                  in_=idx["sdT"][:, ds(it * P, BLK)])

                Gd = sp.tile([P, SUB * HID], BF16, name=f"{tag}_Gd", tag="Gd")
                Gs = sp.tile([P, SUB * HID], BF16, name=f"{tag}_Gs", tag="Gs")
                for j in range(SUB):
                    nc.gpsimd.indirect_dma_start(
                        out=Gd[:, j * HID:(j + 1) * HID], out_offset=None,
                        in_=dst_tab[:],
                        in_offset=IndirectOffsetOnAxis(ap=gdx[:, j:j + 1],
                                                       axis=0))
                    nc.gpsimd.indirect_dma_start(
                        out=Gs[:, j * HID:(j + 1) * HID], out_offset=None,
                        in_=src_tab[:],
                        in_offset=IndirectOffsetOnAxis(ap=gsx[:, j:j + 1],
                                                       axis=0))
                if att_T is None:
                    gax = sp.tile([P, SUB], I32, name=f"{tag}_gax", tag="gax")
                    nc.sync.dma_start(out=gax[:], in_=att_idx[:, ds(it, SUB)])
                    Ga = sp.tile([P, SUB * HID], BF16, name=f"{tag}_Ga",
                                 tag="Ga")
                    for j in range(SUB):
                        nc.gpsimd.indirect_dma_start(
                            out=Ga[:, j * HID:(j + 1) * HID], out_offset=None,
                            in_=att_tab[:],
                            in_offset=IndirectOffsetOnAxis(ap=gax[:, j:j + 1],
                                                           axis=0))

                Sc = sp.tile([P, SUB * HID], BF16, name=f"{tag}_Sc", tag="Sc")
                for g in range(GRP):
                    pk = pp2.tile([P, 512], BF16, name=f"{tag}_pk")
                    for tt in range(4):
                        j = g * 4 + tt
                        nc.tensor.transpose(
                            pk[:HID, tt * P:(tt + 1) * P],
                            Gd[:, j * HID:(j + 1) * HID], K.ident[:])
                        nc.tensor.transpose(
                            pk[HID:, tt * P:(tt + 1) * P],
                            Gs[:, j * HID:(j + 1) * HID], K.ident[:])
                    pks = sp.tile([P, 512], BF16, name=f"{tag}_pks", tag="pks")
                    nc.vector.tensor_copy(pks[:], pk[:])
                    if att_T is None:
                        pe = pp1.tile([HID, 512], BF16, name=f"{tag}_pe")
                        for tt in range(4):
                            j = g * 4 + tt
                            nc.tensor.transpose(
                                pe[:, tt * P:(tt + 1) * P],
                                Ga[:, j * HID:(j + 1) * HID], K.ident[:])
                        att_sb = sp.tile([HID, 512], BF16,
                                         name=f"{tag}_att", tag="att")
                        nc.vector.tensor_copy(att_sb[:], pe[:])
                    else:
                        att_sb = sp.tile([HID, 512], BF16,
                                         name=f"{tag}_attT", tag="att")
                        nc.sync.dma_start(
                            out=att_sb[:],
                            in_=att_T[:, ds(it * P + g * 512, 512)])

                    gu = pp2.tile([P, 512], F32, name=f"{tag}_gu")
                    nc.tensor.matmul(gu[:HID, :],
                                     lhsT=K.Wsd[:, li * HID:(li + 1) * HID],
                                     rhs=pks[:], start=True, stop=False)
                    nc.tensor.matmul(gu[:HID, :],
                                     lhsT=K.Weg[:, li * HID:(li + 1) * HID],
                                     rhs=att_sb[:], start=False, stop=True)
                    nc.tensor.matmul(gu[HID:, :],
                                     lhsT=K.Wdu[HID:, li * HID:(li + 1) * HID],
                                     rhs=pks[HID:, :], start=True, stop=True)
                    zt = sp.tile([HID, 512], BF16, name=f"{tag}_zt",
                                 tag="zt")
                    nc.vector.tensor_scalar(zt[:], gu[:HID, :],
                                            K.bgate[:, li:li + 1], None,
                                            op0=OP.add)
                    sgm = sp.tile([HID, 512], BF16, name=f"{tag}_sgm",
                                  tag="sgm")
                    nc.scalar.activation(sgm[:], zt[:], AF.Sigmoid)
                    gate = sp.tile([HID, 512], BF16, name=f"{tag}_gate",
                                   tag="gate")
                    nc.vector.tensor_mul(gate[:], zt[:], sgm[:])
                    ub = sp.tile([HID, 512], BF16, name=f"{tag}_ub", tag="ub")
                    nc.vector.tensor_scalar(ub[:], gu[HID:, :],
                                            K.bdu[:, li:li + 1], None,
                                            op0=OP.add)
                    msg = sp.tile([HID, 512], BF16, name=f"{tag}_msg",
                                  tag="msg")
                    nc.vector.tensor_mul(msg[:], gate[:], ub[:])
                    mt = pp1.tile([P, 256], BF16, name=f"{tag}_mt")
                    for tt in range(4):
                        nc.tensor.transpose(mt[:, tt * HID:(tt + 1) * HID],
                                            msg[:, tt * P:(tt + 1) * P],
                                            K.ident[:HID, :HID])
                    mts = sp.tile([P, 256], BF16, name=f"{tag}_mts", tag="mts")
                    nc.vector.tensor_copy(mts[:], mt[:])
                    sel = pp1.tile([P, 256], F32, name=f"{tag}_sel")
                    for tt in range(4):
                        j = g * 4 + tt
                        sb2 = pp1.tile([P, P], F32, name=f"{tag}_sb2")
                        nc.tensor.matmul(sb2[:], lhsT=K.ones1[:],
                                         rhs=sdt[:, j * P:(j + 1) * P],
                                         start=True, stop=True)
                        Sm = sp.tile([P, P], BF16, name=f"{tag}_Sm", tag="Sm")
                        nc.vector.tensor_tensor(
                            Sm[:], sdc[:, j:j + 1].to_broadcast([P, P]),
                            sb2[:], op=OP.is_equal)
                        nc.tensor.matmul(sel[:, tt * HID:(tt + 1) * HID],
                                         lhsT=Sm[:],
                                         rhs=mts[:, tt * HID:(tt + 1) * HID],
                                         start=True, stop=True)
                    nc.vector.tensor_copy(Sc[:, g * 256:(g + 1) * 256],
                                          sel[:])
                for j in range(SUB):
                    nc.gpsimd.indirect_dma_start(
                        out=aggr_tab[:],
                        out_offset=IndirectOffsetOnAxis(ap=scx[:, j:j + 1],
                                                        axis=0),
                        in_=Sc[:, j * HID:(j + 1) * HID], in_offset=None)

    # ------------------------------------------------------------------
    # post phase: silu(ln(su(src) + aggr)) + src -> out
    # ------------------------------------------------------------------
    def post_phase(li, rows, src_tab, aggr_tab, out_tab, tag):
        nt = rows // BLK
        with tc.tile_pool(name=f"{tag}_sb", bufs=3) as sp, \
             tc.tile_pool(name=f"{tag}_ps", bufs=2, space="PSUM") as pp:
            with tc.For_i(0, nt * SUB, SUB, staggered_reset=True) as it:
                et = sp.tile([P, SUB * HID], BF16, name=f"{tag}_et", tag="et")
                nc.sync.dma_start(
                    out=et[:].rearrange("p (n d) -> p n d", d=HID),
                    in_=t_rearr(src_tab[ds(it * P, BLK), :]))
                ag = sp.tile([P, SUB * HID], BF16, name=f"{tag}_ag", tag="ag")
                nc.sync.dma_start(
                    out=ag[:].rearrange("p (n d) -> p n d", d=HID),
                    in_=t_rearr(aggr_tab[ds(it * P, BLK), :]))
                ob = sp.tile([P, SUB * HID], BF16, name=f"{tag}_ob", tag="ob")
                for g in range(GRP):
                    pe = pp.tile([HID, 512], BF16, name=f"{tag}_pe")
                    for tt in range(4):
                        j = g * 4 + tt
                        nc.tensor.transpose(pe[:, tt * P:(tt + 1) * P],
                                            et[:, j * HID:(j + 1) * HID],
                                            K.ident[:])
                    pes = sp.tile([HID, 512], BF16, name=f"{tag}_pes",
                                  tag="pes")
                    nc.vector.tensor_copy(pes[:], pe[:])
                    su = pp.tile([HID, 512], F32, name=f"{tag}_su")
                    nc.tensor.matmul(su[:],
                                     lhsT=K.Wsu[:, li * HID:(li + 1) * HID],
                                     rhs=pes[:], start=True, stop=True)
                    sus = sp.tile([HID, 512], BF16, name=f"{tag}_sus",
                                  tag="sus")
                    nc.vector.tensor_scalar(sus[:], su[:],
                                            K.bsu[:, li:li + 1], None,
                                            op0=OP.add)
                    sb_ = pp.tile([P, 256], BF16, name=f"{tag}_sb2")
                    for tt in range(4):
                        nc.tensor.transpose(sb_[:, tt * HID:(tt + 1) * HID],
                                            sus[:, tt * P:(tt + 1) * P],
                                            K.ident[:HID, :HID])
                    tsb = sp.tile([P, 256], F32, name=f"{tag}_tsb", tag="tsb")
                    nc.vector.tensor_add(tsb[:], sb_[:],
                                         ag[:, g * 256:(g + 1) * 256])
                    # LayerNorm over 64-feature groups, row-major
                    t3 = tsb[:].rearrange("p (n d) -> p n d", d=HID)
                    mean = sp.tile([P, 4], F32, name=f"{tag}_mean", tag="mn")
                    nc.vector.tensor_reduce(mean[:], t3, axis=AX.X, op=OP.add)
                    nc.vector.tensor_scalar_mul(mean[:], mean[:], 1.0 / HID)
                    cen = sp.tile([P, 256], F32, name=f"{tag}_cen", tag="cn")
                    nc.vector.tensor_tensor(
                        cen[:].rearrange("p (n d) -> p n d", d=HID), t3,
                        _bcast_mid(mean[:], 4, HID), op=OP.subtract)
                    c3 = cen[:].rearrange("p (n d) -> p n d", d=HID)
                    sqf = sp.tile([P, 256], F32, name=f"{tag}_sqf", tag="sq")
                    nc.vector.tensor_mul(
                        sqf[:].rearrange("p (n d) -> p n d", d=HID), c3, c3)
                    ssq = sp.tile([P, 4], F32, name=f"{tag}_ssq", tag="ssq")
                    nc.vector.tensor_reduce(
                        ssq[:], sqf[:].rearrange("p (n d) -> p n d", d=HID),
                        axis=AX.X, op=OP.add)
                    sdv = sp.tile([P, 4], F32, name=f"{tag}_sdv", tag="sdv")
                    nc.scalar.activation(sdv[:], ssq[:], AF.Sqrt,
                                         bias=K.epsP[:, 0:1],
                                         scale=1.0 / HID)
                    rcp = sp.tile([P, 4], F32, name=f"{tag}_rcp", tag="rcp")
                    nc.vector.reciprocal(rcp[:], sdv[:])
                    nc.vector.tensor_tensor(c3, c3, _bcast_mid(rcp[:], 4, HID),
                                            op=OP.mult)
                    nc.vector.tensor_tensor(c3, c3,
                                            _bcast_row(K.lng[li][:], 4, HID),
                                            op=OP.mult)
                    nc.vector.tensor_tensor(c3, c3,
                                            _bcast_row(K.lnb[li][:], 4, HID),
                                            op=OP.add)
                    sgm = sp.tile([P, 256], BF16, name=f"{tag}_psg",
                                  tag="psg")
                    nc.scalar.activation(sgm[:], cen[:], AF.Sigmoid)
                    slu = sp.tile([P, 256], F32, name=f"{tag}_slu", tag="sl")
                    nc.vector.tensor_mul(slu[:], cen[:], sgm[:])
                    nc.vector.tensor_add(ob[:, g * 256:(g + 1) * 256], slu[:],
                                         et[:, g * 256:(g + 1) * 256])
                nc.sync.dma_start(
                    out=t_rearr(out_tab[ds(it * P, BLK), :]),
                    in_=ob[:].rearrange("p (n d) -> p n d", d=HID))

    # ------------------------------------------------------------------
    # node EGC (gates + chunked AllReduce + replicated post)
    # ------------------------------------------------------------------
    def node_egc(li, lslot, x_in, x_out, e_tab, tag):
        aq = aggr_n[lslot]
        ar_o = ar_out[lslot]
        for q in range(NQ):
            zero_rows(aq[q], NQR)
        for q in range(NQ):
            nidx = {"gd": ins[f"n_gxd{q}"], "gs": ins[f"n_gxs{q}"],
                    "sc": ins[f"n_sc{q}"], "sd": ins[f"n_sd{q}"],
                    "sdT": ins[f"n_sdT{q}"]}
            gate_phase(li, LNQ[q] // P, nidx, x_in, x_in, None, e_tab,
                       ins[f"n_ge{q}"], aq[q], NQR, f"{tag}g{q}")
            nc.gpsimd.collective_compute(
                "AllReduce", OP.add, replica_groups=RG,
                ins=[aq[q][:NQR, :]],
                outs=[ar_o[q * NQR:(q + 1) * NQR, :]])
        post_phase(li, NP_, x_in, ar_o, x_out, f"{tag}p")

    # ------------------------------------------------------------------
    # readout
    # ------------------------------------------------------------------
    def readout(x_fin):
        nt = NP_ // BLK
        with tc.tile_pool(name="ro_sb", bufs=3) as sp, \
             tc.tile_pool(name="ro_acc", bufs=1) as ac, \
             tc.tile_pool(name="ro_ps", bufs=2, space="PSUM") as pp:
            accs = []
            for h in range(GH):
                a = ac.tile([P, HID + 1], F32, name=f"ro_acc{h}")
                nc.gpsimd.memset(a[:], 0.0)
                accs.append(a)
            with tc.For_i(0, nt * SUB, SUB, staggered_reset=True) as it:
                xt = sp.tile([P, SUB * HID], BF16, name="ro_xt", tag="xt")
                nc.sync.dma_start(
                    out=xt[:].rearrange("p (n d) -> p n d", d=HID),
                    in_=t_rearr(x_fin[ds(it * P, BLK), :]))
                bt = sp.tile([P, SUB], F32, name="ro_bt", tag="bt")
                nc.sync.dma_start(out=bt[:], in_=ins["batchf"][:, ds(it, SUB)])
                for j in range(SUB):
                    xa = sp.tile([P, HID + 1], BF16, name="ro_xa", tag="xa")
                    nc.vector.tensor_copy(xa[:, :HID],
                                          xt[:, j * HID:(j + 1) * HID])
                    nc.gpsimd.memset(xa[:, HID:], 1.0)
                    M = sp.tile([P, GH * P], BF16, name="ro_M", tag="M")
                    nc.vector.tensor_tensor(
                        M[:], K.iotaf[:],
                        bt[:, j:j + 1].to_broadcast([P, GH * P]),
                        op=OP.is_equal)
                    for h in range(GH):
                        po = pp.tile([P, HID + 1], F32, name="ro_po")
                        nc.tensor.matmul(po[:], lhsT=M[:, h * P:(h + 1) * P],
                                         rhs=xa[:], start=True, stop=True)
                        nc.vector.tensor_add(accs[h][:], accs[h][:], po[:])
            out_sb = sp.tile([P, GH], F32, name="ro_out")
            for h in range(GH):
                cnt = sp.tile([P, 1], F32, name="ro_cnt", tag="cnt")
                nc.vector.tensor_scalar_max(cnt[:], accs[h][:, HID:], 1.0)
                rc = sp.tile([P, 1], F32, name="ro_rc", tag="rc")
                nc.vector.reciprocal(rc[:], cnt[:])
                pool = sp.tile([P, HID], BF16, name="ro_pool", tag="pool")
                nc.vector.tensor_tensor(pool[:], accs[h][:, :HID],
                                        rc[:].to_broadcast([P, HID]),
                                        op=OP.mult)
                pt = pp.tile([HID, P], BF16, name="ro_pt")
                nc.tensor.transpose(pt[:], pool[:], K.ident[:])
                pts = sp.tile([HID, P], BF16, name="ro_pts", tag="pts")
                nc.vector.tensor_copy(pts[:], pt[:])
                fc = pp.tile([HID, P], F32, name="ro_fc")
                nc.tensor.matmul(fc[:], lhsT=K.Wfc[:], rhs=pts[:],
                                 start=True, stop=True)
                zf = sp.tile([HID, P], F32, name="ro_zf", tag="zf")
                nc.vector.tensor_scalar(zf[:], fc[:], K.bfc[:, 0:1], None,
                                        op0=OP.add)
                sgf = sp.tile([HID, P], BF16, name="ro_sgf", tag="sgf")
                nc.scalar.activation(sgf[:], zf[:], AF.Sigmoid)
                hT = sp.tile([HID, P], BF16, name="ro_hT", tag="hT")
                nc.vector.tensor_mul(hT[:], zf[:], sgf[:])
                oo = pp.tile([P, 1], F32, name="ro_oo")
                nc.tensor.matmul(oo[:], lhsT=hT[:], rhs=K.Wout[:],
                                 start=True, stop=True)
                nc.scalar.activation(out_sb[:, h:h + 1], oo[:], AF.Identity,
                                     bias=K.boutP[:, 0:1])
            for h in range(GH):
                n = min(P, G - h * P)
                if n > 0:
                    nc.sync.dma_start(out=outs["out"][ds(h * P, n), None],
                                      in_=out_sb[:n, h:h + 1])

    # ------------------------------------------------------------------
    # program
    # ------------------------------------------------------------------
    x_emb()
    emb_rbf(ins["edist"], ESP, K.cent_e, meta["gam_e"], K.Wed,
            K.bemb[:, 1:2], NE + 1, e_bufs[0], None, "ee")
    emb_rbf(ins["angp"], LT, K.cent_a, meta["gam_a"], K.Wan,
            K.bemb[:, 2:3], NE + 2, None, aT, "ae")

    tidx = {k: ins["t_" + k] for k in ["gd", "gs", "sc", "sd", "sdT"]}
    send_a2a(0, e_bufs[0])
    for l in range(NLAY):
        zero_rows(aggr_e[l], ESP)
        gate_phase(2 * l, LT // P, tidx, e_bufs[l], recv_b[l], aT, None,
                   None, aggr_e[l], ES, f"eg{l}")
        post_phase(2 * l, ESP, e_bufs[l], aggr_e[l], e_bufs[l + 1],
                   f"ep{l}")
        if l + 1 < NLAY:
            send_a2a(l + 1, e_bufs[l + 1])
        node_egc(2 * l + 1, l, x_bufs[l], x_bufs[l + 1], e_bufs[l + 1],
                 f"na{l}")
    for gg in range(NLAY):
        node_egc(2 * NLAY + gg, NLAY + gg, x_bufs[NLAY + gg],
                 x_bufs[NLAY + gg + 1], e_bufs[NLAY], f"ng{gg}")

    readout(x_bufs[2 * NLAY])

    if meta.get("dbg"):
        for nm, tl in [("dbg_e0", e_bufs[0]), ("dbg_e1", e_bufs[1]),
                       ("dbg_x0", x_bufs[0]), ("dbg_x1", x_bufs[1]),
                       ("dbg_ag", aggr_e[0]), ("dbg_rv", recv_b[0]), ("dbg_sd", send_b[0]),
                       ("dbg_ar", ar_out[0]), ("dbg_xf", x_bufs[2 * NLAY])]:
            if nm in outs:
                nc.sync.dma_start(out=outs[nm], in_=tl[:])

    dram.release()
    K.pool.release()


# ----------------------------------------------------------------------------
# Runner
# ----------------------------------------------------------------------------

_DT = {np.dtype(np.float32): F32, np.dtype(BF): BF16,
       np.dtype(np.int32): I32}


def build_nc(meta, in_map0):
    C = meta["C"]
    nc = bacc.Bacc("TRN2", target_bir_lowering=False, debug=False,
                   num_devices=C)
    ins = {}
    for k, v in in_map0.items():
        t = nc.dram_tensor(k, list(v.shape), _DT[np.dtype(v.dtype)],
                           kind="ExternalInput")
        ins[k] = t[:]
    out_t = nc.dram_tensor("out", [meta["G"]], F32, kind="ExternalOutput")
    outs = {"out": out_t[:]}
    if meta.get("dbg"):
        ESP, NP_, NQR = meta["ESP"], meta["NP"], meta["NQR"]
        CP = meta["C"] * meta["PADM"]
        for nm, shp in [("dbg_e0", [ESP, HID]), ("dbg_e1", [ESP, HID]),
                        ("dbg_x0", [NP_, HID]), ("dbg_x1", [NP_, HID]),
                        ("dbg_ag", [ESP + P, HID]), ("dbg_rv", [CP, HID]), ("dbg_sd", [CP, HID]),
                        ("dbg_ar", [NP_, HID]), ("dbg_xf", [NP_, HID])]:
            outs[nm] = nc.dram_tensor(nm, shp, BF16, kind="ExternalOutput")[:]
    with tile.TileContext(nc) as tc:
        build(tc, outs, ins, meta)
    nc.compile()
    return nc


def kernel(**inputs):
    from concourse import bass_utils
    meta, in_maps = prep(inputs, C=8, BLK=4096)
    nc = build_nc(meta, in_maps[0])
    res = bass_utils.run_bass_kernel_spmd(nc, in_maps,
                                          core_ids=list(range(meta["C"])))
    return np.asarray(res.results[0]["out"], dtype=np.float32)


if __name__ == "__main__":
    pass


# revision 24
# speedup vs baseline: 1.1461x; 1.0026x over previous
"""ALIGNN (nn_ALIGNN_PyG) distributed Trainium2 Bass kernel, 8 NeuronCores.

Sharding (graph-data parallel, comm-minimized):
  - e-rows (line-graph nodes, E) sharded contiguously: E/8 rows per core.
  - Triplets assigned to the owner of their dst edge, processed in dst-sorted
    order, packed into 128-row tiles aligned to segment boundaries (host-side
    padding) so per-tile selection-matmul segment sums never straddle a tile
    and each output row is written by exactly one scatter descriptor.
  - Edge gates need e[src] rows owned by other cores -> AllToAll of the
    unique requested rows (host-computed routing tables).
  - Node-graph edges processed at the core owning the e-row; per-core partial
    aggregates over all N nodes are AllReduced in 4 row-chunks; the node
    post-phase is computed replicated so x stays replicated on all cores.
  - bf16 storage/streams, fp32 PSUM/LN statistics.

All index manipulation (sorting, routing, padding) happens on the host in
numpy; all floating-point math runs on device.
"""

import sys

sys.path.insert(0, "/opt/trn_rl_repo")

import numpy as np
import ml_dtypes

from concourse import bass, bacc, mybir, tile
from concourse.bass import ds, IndirectOffsetOnAxis
from concourse.masks import make_identity

F32 = mybir.dt.float32
BF16 = mybir.dt.bfloat16
I32 = mybir.dt.int32

P = 128
HID = 64
ATOM = 92
EBINS = 40
TBINS = 20
RADIUS = 10.0
NLAY = 4
OOB = 1 << 30
PADSEG = 300.0
EPS = 1e-5
AX = mybir.AxisListType
AF = mybir.ActivationFunctionType
OP = mybir.AluOpType

BF = ml_dtypes.bfloat16


def _bf(x):
    return np.ascontiguousarray(np.asarray(x, dtype=np.float32)).astype(BF)


def _f32(x):
    return np.ascontiguousarray(np.asarray(x, dtype=np.float32))


def _rup(x, m):
    return ((x + m - 1) // m) * m


# ----------------------------------------------------------------------------
# Host preprocessing
# ----------------------------------------------------------------------------

def _segment_slots(d, PP=P):
    """d: sorted int array. Greedy-pack runs of equal values into PP-row tiles
    so no run straddles a tile boundary. Returns slots, per-element tile-local
    run labels (0..PP-1, the run's first slot within its tile; small ints so
    the PE-matmul broadcast of labels is exact even at reduced precision),
    and the tile count."""
    n = len(d)
    if n == 0:
        return np.zeros(0, np.int64), np.zeros(0, np.int64), 0
    bnd = np.flatnonzero(np.diff(d)) + 1
    starts = np.concatenate([[0], bnd]).tolist()
    ends = np.concatenate([bnd, [n]]).tolist()
    slot = np.empty(n, np.int64)
    lab = np.empty(n, np.int64)
    pos = 0
    ar = np.arange(PP)
    for s, e in zip(starts, ends):
        ln = e - s
        assert ln <= PP, f"segment run {ln} > {PP}"
        if (pos % PP) + ln > PP:
            pos = ((pos // PP) + 1) * PP
        slot[s:e] = ar[:ln] + pos
        lab[s:e] = pos % PP
        pos += ln
    return slot, lab, (pos + PP - 1) // PP


def _col128(x):
    """[L] stream -> [128, L//128] with column j = tile j."""
    return np.ascontiguousarray(x.reshape(-1, P).T)


def _pack_stream(dl, extras, dummy):
    """dl: sorted local dst ids. extras: {name: (aligned_array, pad_value)}.
    Masked (pad / non-first-of-segment) rows scatter to the dummy row."""
    slot, lab, nt = _segment_slots(dl)
    L = nt * P
    out = {}
    gd = np.zeros(L, np.int64)
    gd[slot] = dl
    sd = np.full(L, PADSEG, np.float32)
    sd[slot] = lab.astype(np.float32)
    sc = np.full(L, dummy, np.int64)
    if len(dl):
        first = np.ones(len(dl), bool)
        first[1:] = dl[1:] != dl[:-1]
        sc[slot[first]] = dl[first]
    out["gd"], out["sd"], out["sc"] = gd, sd, sc
    for k, (arr, padv) in extras.items():
        a = np.full(L, padv, arr.dtype)
        a[slot] = arr
        out[k] = a
    out["n"] = L
    return out


def _pad_to(st, L, pads):
    for k, padv in pads.items():
        a = st[k]
        if len(a) < L:
            st[k] = np.concatenate([a, np.full(L - len(a), padv, a.dtype)])
    st["n"] = L


_BASE_PADS = {"gd": np.int64(0), "sd": np.float32(PADSEG)}


def prep(inputs, C=8, BLK=4096):
    x_atom = _f32(inputs["x_atom"])
    edge_dist = _f32(inputs["edge_dist"])
    angle_cos = _f32(inputs["angle_cos"])
    params = inputs["params"]
    ei = np.asarray(inputs["edge_index"]).astype(np.int64)
    lg = np.asarray(inputs["lg_edge_index"]).astype(np.int64)
    batch = np.asarray(inputs["batch"]).astype(np.int64)
    G = int(np.asarray(inputs["num_graphs"]))

    N, E = x_atom.shape[0], edge_dist.shape[0]
    assert E % C == 0
    ES = E // C
    NP_ = _rup(N, BLK)
    ESP = _rup(ES, BLK)
    NQ = 4
    assert N % NQ == 0
    NQR = N // NQ

    meta = dict(C=C, N=N, E=E, G=G, ES=ES, NP=NP_, ESP=ESP, NQ=NQ, NQR=NQR,
                BLK=BLK)

    # ---- triplet (edge-EGC) streams ----------------------------------------
    src_t, dst_t = lg[0], lg[1]
    own_t = dst_t // ES
    trip = []
    for c in range(C):
        m = np.flatnonzero(own_t == c)
        o = m[np.argsort(dst_t[m], kind="stable")]
        dl = dst_t[o] - c * ES
        trip.append(_pack_stream(dl, {"sg": (src_t[o], np.int64(-1)),
                                      "ang": (angle_cos[o], np.float32(0))},
                                 ESP))
    LT = _rup(max(t["n"] for t in trip), BLK)
    for t in trip:
        _pad_to(t, LT, dict(_BASE_PADS, sc=np.int64(ESP), sg=np.int64(-1),
                            ang=np.float32(0)))
    meta["LT"] = LT

    # ---- AllToAll routing for e[src] rows ----------------------------------
    uniq = [[None] * C for _ in range(C)]
    for c in range(C):
        sgl = trip[c]["sg"]
        for s in range(C):
            sel = sgl[(sgl >= s * ES) & (sgl < (s + 1) * ES)]
            uniq[s][c] = np.unique(sel)
    PADM = max(max(len(uniq[s][c]) for c in range(C)) for s in range(C))
    PADM = max(PADM, 1)
    PADM = _rup(PADM, BLK // C) if (BLK % C == 0) else _rup(PADM, P)
    while (C * PADM) % BLK != 0:
        PADM += P
    meta["PADM"] = PADM
    send_idx = np.zeros((C, C * PADM), np.int64)
    for s in range(C):
        for c in range(C):
            ids = uniq[s][c] - s * ES
            send_idx[s, c * PADM:c * PADM + len(ids)] = ids
    for c in range(C):
        sgl = trip[c]["sg"]
        gs = np.zeros(LT, np.int64)
        for s in range(C):
            msk = (sgl >= s * ES) & (sgl < (s + 1) * ES)
            gs[msk] = s * PADM + np.searchsorted(uniq[s][c], sgl[msk])
        gs[sgl < 0] = 0
        trip[c]["gs"] = gs

    # ---- node-EGC streams, quartered for chunked AllReduce -----------------
    src_n, dst_n = ei[0], ei[1]
    node = []
    for c in range(C):
        j0, j1 = c * ES, (c + 1) * ES
        dd = dst_n[j0:j1]
        ss = src_n[j0:j1]
        qs = []
        for q in range(NQ):
            m = np.flatnonzero((dd >= q * NQR) & (dd < (q + 1) * NQR))
            o = m[np.argsort(dd[m], kind="stable")]
            dl = dd[o] - q * NQR
            qs.append(_pack_stream(dl, {"ge": (o.astype(np.int64), np.int64(0)),
                                        "gxs": (ss[o], np.int64(0)),
                                        "gxd": (dd[o], np.int64(0))}, NQR))
        node.append(qs)
    LNQ = [_rup(max(max(node[c][q]["n"] for c in range(C)), BLK), BLK)
           for q in range(NQ)]
    for c in range(C):
        for q in range(NQ):
            _pad_to(node[c][q], LNQ[q],
                    dict(_BASE_PADS, sc=np.int64(NQR), ge=np.int64(0),
                         gxs=np.int64(0), gxd=np.int64(0)))
    meta["LNQ"] = LNQ

    # ---- weights -----------------------------------------------------------
    def lin(p):
        return _f32(p["w"]), _f32(p["b"])

    egcs = []
    for l in range(NLAY):
        egcs.append(params["alignn"][l]["edge"])
        egcs.append(params["alignn"][l]["node"])
    egcs.extend(params["gcn"])
    NE = len(egcs)
    meta["NE"] = NE

    Wsd = np.zeros((P, NE * HID), np.float32)
    Weg = np.zeros((HID, NE * HID), np.float32)
    Wdu = np.zeros((P, NE * HID), np.float32)
    Wsu = np.zeros((HID, NE * HID), np.float32)
    bgate = np.zeros((HID, NE), np.float32)
    bdu = np.zeros((HID, NE), np.float32)
    bsu = np.zeros((HID, NE), np.float32)
    lngr = np.zeros((NE + 3, HID), np.float32)
    lnbr = np.zeros((NE + 3, HID), np.float32)
    for i, p in enumerate(egcs):
        sw, sb_ = lin(p["sg"])
        dw, db = lin(p["dg"])
        ew, eb = lin(p["eg"])
        uw, ub = lin(p["du"])
        tw, tb = lin(p["su"])
        Wsd[:HID, i * HID:(i + 1) * HID] = sw
        Wsd[HID:, i * HID:(i + 1) * HID] = dw
        Weg[:, i * HID:(i + 1) * HID] = ew
        Wdu[HID:, i * HID:(i + 1) * HID] = uw
        Wsu[:, i * HID:(i + 1) * HID] = tw
        bgate[:, i] = sb_ + db + eb
        bdu[:, i] = ub
        bsu[:, i] = tb
        lngr[i] = _f32(p["ln_g"])
        lnbr[i] = _f32(p["ln_b"])

    embs = [params["atom_emb"], params["edge_emb"], params["angle_emb"]]
    Wat = np.zeros((ATOM, HID), np.float32)
    Wed = np.zeros((EBINS, HID), np.float32)
    Wan = np.zeros((TBINS, HID), np.float32)
    bemb = np.zeros((HID, 3), np.float32)
    for i, p in enumerate(embs):
        w, b = lin(p)
        [Wat, Wed, Wan][i][:, :] = w
        bemb[:, i] = b
        lngr[NE + i] = _f32(p["ln_g"])
        lnbr[NE + i] = _f32(p["ln_b"])

    Wfc, bfc = lin(params["fc"])
    Wout, bout = lin(params["out"])

    cent_e = np.linspace(0.0, RADIUS, EBINS).astype(np.float32)
    gam_e = 1.0 / (cent_e[1] - cent_e[0]) ** 2
    cent_a = np.linspace(-1.0, 1.0, TBINS).astype(np.float32)
    gam_a = 1.0 / (cent_a[1] - cent_a[0]) ** 2
    meta["gam_e"], meta["gam_a"] = float(gam_e), float(gam_a)
    meta["bout"] = float(bout[0])
    meta["GH"] = _rup(G, P) // P

    xa_pad = np.zeros((NP_, ATOM), np.float32)
    xa_pad[:N] = x_atom
    bat_pad = np.full(NP_, 2.0e6, np.float32)
    bat_pad[:N] = batch.astype(np.float32)

    shared = dict(
        x_atom=xa_pad,
        batchf=_col128(bat_pad),
        Wsd=_bf(Wsd), Weg=_bf(Weg), Wdu=_bf(Wdu), Wsu=_bf(Wsu),
        bgate=bgate, bdu=bdu, bsu=bsu,
        lngr=lngr, lnbr=lnbr,
        lngc=np.ascontiguousarray(lngr.T), lnbc=np.ascontiguousarray(lnbr.T),
        Wat=_bf(Wat), Wed=_bf(Wed), Wan=_bf(Wan), bemb=bemb,
        Wfc=_bf(Wfc), bfc=_f32(bfc).reshape(HID, 1),
        Wout=_bf(Wout).reshape(HID, 1),
        cent_e=cent_e.reshape(EBINS, 1), cent_a=cent_a.reshape(TBINS, 1),
    )

    in_maps = []
    for c in range(C):
        ed_pad = np.zeros(ESP, np.float32)
        ed_pad[:ES] = edge_dist[c * ES:(c + 1) * ES]
        t = trip[c]
        m = dict(shared)
        m["edist"] = ed_pad
        m["angp"] = t["ang"]
        m["t_gd"] = _col128(t["gd"].astype(np.int32))
        m["t_gs"] = _col128(t["gs"].astype(np.int32))
        m["t_sc"] = _col128(t["sc"].astype(np.int32))
        m["t_sd"] = _col128(t["sd"])
        m["t_sdT"] = t["sd"].reshape(1, -1)
        m["sendix"] = _col128(send_idx[c].astype(np.int32))
        for q in range(NQ):
            nq = node[c][q]
            m[f"n_ge{q}"] = _col128(nq["ge"].astype(np.int32))
            m[f"n_gxs{q}"] = _col128(nq["gxs"].astype(np.int32))
            m[f"n_gxd{q}"] = _col128(nq["gxd"].astype(np.int32))
            m[f"n_sc{q}"] = _col128(nq["sc"].astype(np.int32))
            m[f"n_sd{q}"] = _col128(nq["sd"])
            m[f"n_sdT{q}"] = nq["sd"].reshape(1, -1)
        in_maps.append(m)

    return meta, in_maps


# ----------------------------------------------------------------------------
# Device kernel
# ----------------------------------------------------------------------------

def _bcast_mid(ap2d, nsub, inner):
    """[128, k] AP -> [128, (1,k)... wait: build [p, nsub, inner] view with the
    given free pattern pairs."""
    return bass.AP(ap2d.tensor, ap2d.offset, [ap2d.ap[0], (1, nsub),
                                              (0, inner)])


def _bcast_row(ap2d, nsub, inner):
    """[128, inner] AP -> [p, nsub(bcast), inner]."""
    return bass.AP(ap2d.tensor, ap2d.offset, [ap2d.ap[0], (0, nsub),
                                              (1, inner)])


class Consts:
    pass


def _load_consts(tc, nc, ins, meta):
    K = Consts()
    cp = tc.alloc_tile_pool(name="consts", bufs=1)
    K.pool = cp

    def sb(name):
        a = ins[name]
        t = cp.tile(list(a.shape), a.dtype, name="c_" + name)
        nc.sync.dma_start(out=t[:], in_=a[:])
        return t

    for nm in ["Wsd", "Weg", "Wdu", "Wsu", "bgate", "bdu", "bsu",
               "Wat", "Wed", "Wan", "bemb", "Wfc", "bfc", "Wout",
               "cent_e", "cent_a", "lngc", "lnbc"]:
        setattr(K, nm, sb(nm))

    NE = meta["NE"]
    K.lng = []
    K.lnb = []
    for i in range(NE):
        gr = cp.tile([1, HID], F32, name=f"lngr{i}")
        nc.sync.dma_start(out=gr[:], in_=ins["lngr"][i:i + 1, :])
        br = cp.tile([1, HID], F32, name=f"lnbr{i}")
        nc.sync.dma_start(out=br[:], in_=ins["lnbr"][i:i + 1, :])
        g = cp.tile([P, HID], F32, name=f"lng{i}")
        b = cp.tile([P, HID], F32, name=f"lnb{i}")
        nc.gpsimd.partition_broadcast(g[:], gr[:])
        nc.gpsimd.partition_broadcast(b[:], br[:])
        K.lng.append(g)
        K.lnb.append(b)

    K.ident = cp.tile([P, P], BF16, name="identbf")
    make_identity(nc, K.ident[:])
    K.identf = cp.tile([P, P], F32, name="identf")
    make_identity(nc, K.identf[:])

    K.ones1 = cp.tile([1, P], F32, name="ones1")
    nc.gpsimd.memset(K.ones1[:], 1.0)

    K.stS = cp.tile([P, 1], BF16, name="stS")
    nc.gpsimd.memset(K.stS[:], 0.0)
    nc.gpsimd.memset(K.stS[:HID, 0:1], 1.0)
    K.stQ = cp.tile([P, 1], BF16, name="stQ")
    nc.gpsimd.memset(K.stQ[:], 0.0)
    nc.gpsimd.memset(K.stQ[HID:, 0:1], 1.0)

    K.epsP = cp.tile([P, 1], F32, name="epsP")
    nc.gpsimd.memset(K.epsP[:], EPS)
    K.boutP = cp.tile([P, 1], F32, name="boutP")
    nc.gpsimd.memset(K.boutP[:], float(meta["bout"]))

    GH = meta["GH"]
    it = cp.tile([P, GH * P], I32, name="iotai")
    nc.gpsimd.iota(it[:], pattern=[[1, GH * P]], base=0, channel_multiplier=0)
    K.iotaf = cp.tile([P, GH * P], F32, name="iotaf")
    nc.vector.tensor_copy(K.iotaf[:], it[:])
    return K


def build(tc, outs, ins, meta):
    nc = tc.nc
    C, BLK = meta["C"], meta["BLK"]
    ES, ESP, NP_, LT = meta["ES"], meta["ESP"], meta["NP"], meta["LT"]
    NQ, NQR, LNQ = meta["NQ"], meta["NQR"], meta["LNQ"]
    PADM, NE, G, GH = meta["PADM"], meta["NE"], meta["G"], meta["GH"]
    N = meta["N"]
    SUB = BLK // P
    GRP = BLK // 512
    RG = [list(range(C))]

    K = _load_consts(tc, nc, ins, meta)

    dram = tc.alloc_tile_pool(name="dram", bufs=1, space="DRAM")
    e_bufs = [dram.tile([ESP, HID], BF16, name=f"e{l}")
              for l in range(NLAY + 1)]
    x_bufs = [dram.tile([NP_, HID], BF16, name=f"x{l}")
              for l in range(2 * NLAY + 1)]
    aT = dram.tile([HID, LT], BF16, name="aT")
    send_b = [dram.tile([C * PADM, HID], BF16, name=f"send{l}")
              for l in range(NLAY)]
    recv_b = [dram.tile([C * PADM, HID], BF16, name=f"recv{l}")
              for l in range(NLAY)]
    aggr_e = [dram.tile([ESP + P, HID], BF16, name=f"aggre{l}")
              for l in range(NLAY)]
    aggr_n = [[dram.tile([NQR + P, HID], BF16, name=f"aggrn{l}_{q}")
               for q in range(NQ)] for l in range(2 * NLAY)]
    ar_out = [dram.tile([NP_, HID], BF16, name=f"arout{l}")
              for l in range(2 * NLAY)]
    zeros_d = dram.tile([BLK, HID], BF16, name="zerod")

    with tc.tile_pool(name="zinit", bufs=1) as zp:
        zt = zp.tile([P, SUB * HID], BF16)
        nc.gpsimd.memset(zt[:], 0.0)
        nc.sync.dma_start(
            out=zeros_d[:].rearrange("(n p) d -> p n d", p=P), in_=zt[:])

    def zero_rows(tab, rows):
        r = 0
        while r < rows:
            n = min(BLK, rows - r)
            nc.sync.dma_start(out=tab[r:r + n, :], in_=zeros_d[0:n, :])
            r += n

    # ar_out pad rows [N, NP_) are never written by the AllReduce: zero them
    # once so the replicated node post-phase can't read NaNs into x pads.
    if NP_ > N:
        for l in range(2 * NLAY):
            nc.sync.dma_start(out=ar_out[l][N:NP_, :],
                              in_=zeros_d[0:NP_ - N, :])

    def t_rearr(ap):
        return ap.rearrange("(n p) d -> p n d", p=P)

    # ------------------------------------------------------------------
    # shared LN helper in transposed [HID, 512] layout (embeddings)
    # ------------------------------------------------------------------
    def ln_T(sp, pq, ps_in, bias_ap, lnci, tag):
        """ps_in: PSUM [HID,512] f32 = pre-LN linear output (no bias yet).
        Returns SBUF bf16 [HID,512] tile of silu(ln(x+b))."""
        xb = sp.tile([P, 512], BF16, name=f"{tag}_xb", tag=f"{tag}xb")
        nc.vector.tensor_scalar(xb[:HID, :], ps_in[:], bias_ap, None,
                                op0=OP.add)
        nc.vector.tensor_mul(xb[HID:, :], xb[:HID, :], xb[:HID, :])
        st = pq.tile([1, 512], F32, name=f"{tag}_st", tag=f"{tag}st")
        nc.tensor.matmul(st[:], lhsT=K.stS[:], rhs=xb[:], start=True,
                         stop=True)
        mean = sp.tile([1, 512], F32, name=f"{tag}_mean", tag=f"{tag}mn")
        nc.vector.tensor_scalar_mul(mean[:], st[:], 1.0 / HID)
        stq = pq.tile([1, 512], F32, name=f"{tag}_stq", tag=f"{tag}sq2")
        nc.tensor.matmul(stq[:], lhsT=K.stQ[:], rhs=xb[:], start=True,
                         stop=True)
        var = sp.tile([1, 512], F32, name=f"{tag}_var", tag=f"{tag}vr")
        nc.vector.tensor_scalar_mul(var[:], stq[:], 1.0 / HID)
        msq = sp.tile([1, 512], F32, name=f"{tag}_msq", tag=f"{tag}mq")
        nc.vector.tensor_mul(msq[:], mean[:], mean[:])
        nc.vector.tensor_sub(var[:], var[:], msq[:])
        sdv = sp.tile([1, 512], F32, name=f"{tag}_sdv", tag=f"{tag}sd")
        nc.scalar.activation(sdv[:], var[:], AF.Sqrt, bias=K.epsP[0:1, 0:1])
        rcp = sp.tile([1, 512], F32, name=f"{tag}_rcp", tag=f"{tag}rc")
        nc.vector.reciprocal(rcp[:], sdv[:])
        mrb = sp.tile([HID, 1024], F32, name=f"{tag}_mrb", tag=f"{tag}mb")
        nc.gpsimd.partition_broadcast(mrb[:, :512], mean[:])
        nc.gpsimd.partition_broadcast(mrb[:, 512:], rcp[:])
        t1 = sp.tile([HID, 512], F32, name=f"{tag}_t1", tag=f"{tag}t1")
        nc.vector.tensor_sub(t1[:], xb[:HID, :], mrb[:, :512])
        nc.vector.tensor_mul(t1[:], t1[:], mrb[:, 512:])
        nc.vector.tensor_scalar(t1[:], t1[:], K.lngc[:, lnci:lnci + 1],
                                K.lnbc[:, lnci:lnci + 1], op0=OP.mult,
                                op1=OP.add)
        sg_t = sp.tile([HID, 512], BF16, name=f"{tag}_sg", tag=f"{tag}sg")
        nc.scalar.activation(sg_t[:], t1[:], AF.Sigmoid)
        sl = sp.tile([HID, 512], BF16, name=f"{tag}_sl", tag=f"{tag}sl")
        nc.vector.tensor_mul(sl[:], t1[:], sg_t[:])
        return sl

    def rows_out(sp, pq, sl, out_tab, r0, tag):
        """Transpose [HID,512] bf16 back to rows and DMA to out_tab[r0:r0+512]."""
        tb = pq.tile([P, 256], BF16, name=f"{tag}_tb", tag=f"{tag}tb")
        for tt in range(4):
            nc.tensor.transpose(tb[:, tt * HID:(tt + 1) * HID],
                                sl[:, tt * P:(tt + 1) * P],
                                K.ident[:HID, :HID])
        ro = sp.tile([P, 256], BF16, name=f"{tag}_ro", tag=f"{tag}ro")
        nc.vector.tensor_copy(ro[:], tb[:])
        nc.sync.dma_start(out=t_rearr(out_tab[r0:r0 + 512, :])
                          if isinstance(r0, int)
                          else t_rearr(out_tab[ds(r0, 512), :]), in_=ro[:])

    # ------------------------------------------------------------------
    # embeddings
    # ------------------------------------------------------------------
    def emb_rbf(dist_in, L, cent, gam, Wt, bias_ap, lnci, out_rows, out_T,
                tag):
        nbins = cent.shape[0]
        nt = L // BLK
        with tc.tile_pool(name=f"{tag}_sb", bufs=3) as sp, \
             tc.tile_pool(name=f"{tag}_ps", bufs=2, space="PSUM") as pp, \
             tc.tile_pool(name=f"{tag}_pq", bufs=1, space="PSUM") as pq:
            with tc.For_i(0, nt * SUB, SUB, staggered_reset=True) as it:
                dchunk = sp.tile([1, BLK], F32, name=f"{tag}_dch")
                nc.sync.dma_start(out=dchunk[:],
                                  in_=dist_in[None, ds(it * P, BLK)])
                for g in range(GRP):
                    gsl = slice(g * 512, (g + 1) * 512)
                    dbc = sp.tile([nbins, 512], F32, name=f"{tag}_dbc",
                                  tag="dbc")
                    nc.gpsimd.partition_broadcast(dbc[:], dchunk[:, gsl])
                    nc.vector.tensor_scalar(dbc[:], dbc[:], cent[:, 0:1],
                                            None, op0=OP.subtract)
                    sqv = sp.tile([nbins, 512], F32, name=f"{tag}_sqv",
                                  tag="sqv")
                    nc.vector.tensor_mul(sqv[:], dbc[:], dbc[:])
                    rbf = sp.tile([nbins, 512], BF16, name=f"{tag}_rbf",
                                  tag="rbf")
                    nc.scalar.activation(rbf[:], sqv[:], AF.Exp, scale=-gam)
                    ps = pp.tile([HID, 512], F32, name=f"{tag}_ps0")
                    nc.tensor.matmul(ps[:], lhsT=Wt[:], rhs=rbf[:],
                                     start=True, stop=True)
                    sl = ln_T(sp, pq, ps, bias_ap, lnci, tag)
                    if out_T is not None:
                        nc.sync.dma_start(
                            out=out_T[:, ds(it * P + g * 512, 512)],
                            in_=sl[:])
                    if out_rows is not None:
                        rows_out(sp, pq, sl, out_rows, it * P + g * 512, tag)

    def x_emb():
        nt = NP_ // BLK
        xa = ins["x_atom"]
        with tc.tile_pool(name="xe_sb", bufs=3) as sp, \
             tc.tile_pool(name="xe_ps", bufs=2, space="PSUM") as pp, \
             tc.tile_pool(name="xe_pq", bufs=1, space="PSUM") as pq:
            with tc.For_i(0, nt * SUB, SUB, staggered_reset=True) as it:
                for g in range(GRP):
                    xt = sp.tile([P, 4 * ATOM], F32, name="xe_xt", tag="xt")
                    nc.sync.dma_start(
                        out=xt[:].rearrange("p (n d) -> p n d", d=ATOM),
                        in_=t_rearr(xa[ds(it * P + g * 512, 512), :]))
                    tp = pp.tile([ATOM, 512], F32, name="xe_tp")
                    for tt in range(4):
                        nc.tensor.transpose(tp[:, tt * P:(tt + 1) * P],
                                            xt[:, tt * ATOM:(tt + 1) * ATOM],
                                            K.identf[:])
                    tps = sp.tile([ATOM, 512], BF16, name="xe_tps", tag="tps")
                    nc.vector.tensor_copy(tps[:], tp[:])
                    ps = pp.tile([HID, 512], F32, name="xe_ps0")
                    nc.tensor.matmul(ps[:], lhsT=K.Wat[:], rhs=tps[:],
                                     start=True, stop=True)
                    sl = ln_T(sp, pq, ps, K.bemb[:, 0:1], NE + 0, "xe")
                    rows_out(sp, pq, sl, x_bufs[0], it * P + g * 512, "xe")

    # ------------------------------------------------------------------
    # AllToAll send gather
    # ------------------------------------------------------------------
    def send_a2a(l, e_src):
        nt = (C * PADM) // BLK
        with tc.tile_pool(name="snd_sb", bufs=6) as sp:
            with tc.For_i(0, nt * SUB, SUB, staggered_reset=True) as it:
                six = sp.tile([P, SUB], I32, name="snd_six")
                nc.sync.dma_start(out=six[:],
                                  in_=ins["sendix"][:, ds(it, SUB)])
                gt = sp.tile([P, SUB * HID], BF16, name="snd_gt")
                for j in range(SUB):
                    nc.gpsimd.indirect_dma_start(
                        out=gt[:, j * HID:(j + 1) * HID], out_offset=None,
                        in_=e_src[:],
                        in_offset=IndirectOffsetOnAxis(ap=six[:, j:j + 1],
                                                       axis=0))
                nc.sync.dma_start(
                    out=t_rearr(send_b[l][ds(it * P, BLK), :]), in_=gt[:])
        nc.gpsimd.collective_compute(
            "AllToAll", OP.bypass, replica_groups=RG,
            ins=[send_b[l][:]], outs=[recv_b[l][:]])

    # ------------------------------------------------------------------
    # gate + scatter phase
    # ------------------------------------------------------------------
    def gate_phase(li, n_tiles, idx, dst_tab, src_tab, att_T, att_tab,
                   att_idx, aggr_tab, aggr_rows, tag):
        with tc.tile_pool(name=f"{tag}_sb", bufs=4) as sp, \
             tc.tile_pool(name=f"{tag}_p2", bufs=2, space="PSUM") as pp2, \
             tc.tile_pool(name=f"{tag}_p1", bufs=1, space="PSUM") as pp1:
            with tc.For_i(0, n_tiles, SUB, staggered_reset=True) as it:
                gdx = sp.tile([P, SUB], I32, name=f"{tag}_gdx", tag="gdx")
                nc.sync.dma_start(out=gdx[:], in_=idx["gd"][:, ds(it, SUB)])
                gsx = sp.tile([P, SUB], I32, name=f"{tag}_gsx", tag="gsx")
                nc.sync.dma_start(out=gsx[:], in_=idx["gs"][:, ds(it, SUB)])
                scx = sp.tile([P, SUB], I32, name=f"{tag}_scx", tag="scx")
                nc.sync.dma_start(out=scx[:], in_=idx["sc"][:, ds(it, SUB)])
                sdc = sp.tile([P, SUB], F32, name=f"{tag}_sdc", tag="sdc")
                nc.sync.dma_start(out=sdc[:], in_=idx["sd"][:, ds(it, SUB)])
                sdt = sp.tile([1, BLK], F32, name=f"{tag}_sdt", tag="sdt")
                nc.sync.dma_start(out=sdt[:],
                                  in_=idx["sdT"][:, ds(it * P, BLK)])

                Gd = sp.tile([P, SUB * HID], BF16, name=f"{tag}_Gd", tag="Gd")
                Gs = sp.tile([P, SUB * HID], BF16, name=f"{tag}_Gs", tag="Gs")
                for j in range(SUB):
                    nc.gpsimd.indirect_dma_start(
                        out=Gd[:, j * HID:(j + 1) * HID], out_offset=None,
                        in_=dst_tab[:],
                        in_offset=IndirectOffsetOnAxis(ap=gdx[:, j:j + 1],
                                                       axis=0))
                    nc.gpsimd.indirect_dma_start(
                        out=Gs[:, j * HID:(j + 1) * HID], out_offset=None,
                        in_=src_tab[:],
                        in_offset=IndirectOffsetOnAxis(ap=gsx[:, j:j + 1],
                                                       axis=0))
                if att_T is None:
                    gax = sp.tile([P, SUB], I32, name=f"{tag}_gax", tag="gax")
                    nc.sync.dma_start(out=gax[:], in_=att_idx[:, ds(it, SUB)])
                    Ga = sp.tile([P, SUB * HID], BF16, name=f"{tag}_Ga",
                                 tag="Ga")
                    for j in range(SUB):
                        nc.gpsimd.indirect_dma_start(
                            out=Ga[:, j * HID:(j + 1) * HID], out_offset=None,
                            in_=att_tab[:],
                            in_offset=IndirectOffsetOnAxis(ap=gax[:, j:j + 1],
                                                           axis=0))

                Sc = sp.tile([P, SUB * HID], BF16, name=f"{tag}_Sc", tag="Sc")
                for g in range(GRP):
                    pk = pp2.tile([P, 512], BF16, name=f"{tag}_pk")
                    for tt in range(4):
                        j = g * 4 + tt
                        nc.tensor.transpose(
                            pk[:HID, tt * P:(tt + 1) * P],
                            Gd[:, j * HID:(j + 1) * HID], K.ident[:])
                        nc.tensor.transpose(
                            pk[HID:, tt * P:(tt + 1) * P],
                            Gs[:, j * HID:(j + 1) * HID], K.ident[:])
                    pks = sp.tile([P, 512], BF16, name=f"{tag}_pks", tag="pks")
                    nc.vector.tensor_copy(pks[:], pk[:])
                    if att_T is None:
                        pe = pp1.tile([HID, 512], BF16, name=f"{tag}_pe")
                        for tt in range(4):
                            j = g * 4 + tt
                            nc.tensor.transpose(
                                pe[:, tt * P:(tt + 1) * P],
                                Ga[:, j * HID:(j + 1) * HID], K.ident[:])
                        att_sb = sp.tile([HID, 512], BF16,
                                         name=f"{tag}_att", tag="att")
                        nc.vector.tensor_copy(att_sb[:], pe[:])
                    else:
                        att_sb = sp.tile([HID, 512], BF16,
                                         name=f"{tag}_attT", tag="att")
                        nc.sync.dma_start(
                            out=att_sb[:],
                            in_=att_T[:, ds(it * P + g * 512, 512)])

                    gu = pp2.tile([P, 512], F32, name=f"{tag}_gu")
                    nc.tensor.matmul(gu[:HID, :],
                                     lhsT=K.Wsd[:, li * HID:(li + 1) * HID],
                                     rhs=pks[:], start=True, stop=False)
                    nc.tensor.matmul(gu[:HID, :],
                                     lhsT=K.Weg[:, li * HID:(li + 1) * HID],
                                     rhs=att_sb[:], start=False, stop=True)
                    nc.tensor.matmul(gu[HID:, :],
                                     lhsT=K.Wdu[HID:, li * HID:(li + 1) * HID],
                                     rhs=pks[HID:, :], start=True, stop=True)
                    zt = sp.tile([HID, 512], BF16, name=f"{tag}_zt",
                                 tag="zt")
                    nc.vector.tensor_scalar(zt[:], gu[:HID, :],
                                            K.bgate[:, li:li + 1], None,
                                            op0=OP.add)
                    sgm = sp.tile([HID, 512], BF16, name=f"{tag}_sgm",
                                  tag="sgm")
                    nc.scalar.activation(sgm[:], zt[:], AF.Sigmoid)
                    gate = sp.tile([HID, 512], BF16, name=f"{tag}_gate",
                                   tag="gate")
                    nc.vector.tensor_mul(gate[:], zt[:], sgm[:])
                    ub = sp.tile([HID, 512], BF16, name=f"{tag}_ub", tag="ub")
                    nc.vector.tensor_scalar(ub[:], gu[HID:, :],
                                            K.bdu[:, li:li + 1], None,
                                            op0=OP.add)
                    msg = sp.tile([HID, 512], BF16, name=f"{tag}_msg",
                                  tag="msg")
                    nc.vector.tensor_mul(msg[:], gate[:], ub[:])
                    mt = pp1.tile([P, 256], BF16, name=f"{tag}_mt")
                    for tt in range(4):
                        nc.tensor.transpose(mt[:, tt * HID:(tt + 1) * HID],
                                            msg[:, tt * P:(tt + 1) * P],
                                            K.ident[:HID, :HID])
                    mts = sp.tile([P, 256], BF16, name=f"{tag}_mts", tag="mts")
                    nc.vector.tensor_copy(mts[:], mt[:])
                    sel = pp1.tile([P, 256], F32, name=f"{tag}_sel")
                    for tt in range(4):
                        j = g * 4 + tt
                        sb2 = pp1.tile([P, P], F32, name=f"{tag}_sb2")
                        nc.tensor.matmul(sb2[:], lhsT=K.ones1[:],
                                         rhs=sdt[:, j * P:(j + 1) * P],
                                         start=True, stop=True)
                        Sm = sp.tile([P, P], BF16, name=f"{tag}_Sm", tag="Sm")
                        nc.vector.tensor_tensor(
                            Sm[:], sdc[:, j:j + 1].to_broadcast([P, P]),
                            sb2[:], op=OP.is_equal)
                        nc.tensor.matmul(sel[:, tt * HID:(tt + 1) * HID],
                                         lhsT=Sm[:],
                                         rhs=mts[:, tt * HID:(tt + 1) * HID],
                                         start=True, stop=True)
                    nc.vector.tensor_copy(Sc[:, g * 256:(g + 1) * 256],
                                          sel[:])
                for j in range(SUB):
                    nc.gpsimd.indirect_dma_start(
                        out=aggr_tab[:],
                        out_offset=IndirectOffsetOnAxis(ap=scx[:, j:j + 1],
                                                        axis=0),
                        in_=Sc[:, j * HID:(j + 1) * HID], in_offset=None)

    # ------------------------------------------------------------------
    # post phase: silu(ln(su(src) + aggr)) + src -> out
    # ------------------------------------------------------------------
    def post_phase(li, rows, src_tab, aggr_tab, out_tab, tag):
        nt = rows // BLK
        with tc.tile_pool(name=f"{tag}_sb", bufs=4) as sp, \
             tc.tile_pool(name=f"{tag}_ps", bufs=2, space="PSUM") as pp:
            with tc.For_i(0, nt * SUB, SUB, staggered_reset=True) as it:
                et = sp.tile([P, SUB * HID], BF16, name=f"{tag}_et", tag="et")
                nc.sync.dma_start(
                    out=et[:].rearrange("p (n d) -> p n d", d=HID),
                    in_=t_rearr(src_tab[ds(it * P, BLK), :]))
                ag = sp.tile([P, SUB * HID], BF16, name=f"{tag}_ag", tag="ag")
                nc.sync.dma_start(
                    out=ag[:].rearrange("p (n d) -> p n d", d=HID),
                    in_=t_rearr(aggr_tab[ds(it * P, BLK), :]))
                ob = sp.tile([P, SUB * HID], BF16, name=f"{tag}_ob", tag="ob")
                for g in range(GRP):
                    pe = pp.tile([HID, 512], BF16, name=f"{tag}_pe")
                    for tt in range(4):
                        j = g * 4 + tt
                        nc.tensor.transpose(pe[:, tt * P:(tt + 1) * P],
                                            et[:, j * HID:(j + 1) * HID],
                                            K.ident[:])
                    pes = sp.tile([HID, 512], BF16, name=f"{tag}_pes",
                                  tag="pes")
                    nc.vector.tensor_copy(pes[:], pe[:])
                    su = pp.tile([HID, 512], F32, name=f"{tag}_su")
                    nc.tensor.matmul(su[:],
                                     lhsT=K.Wsu[:, li * HID:(li + 1) * HID],
                                     rhs=pes[:], start=True, stop=True)
                    sus = sp.tile([HID, 512], BF16, name=f"{tag}_sus",
                                  tag="sus")
                    nc.vector.tensor_scalar(sus[:], su[:],
                                            K.bsu[:, li:li + 1], None,
                                            op0=OP.add)
                    sb_ = pp.tile([P, 256], BF16, name=f"{tag}_sb2")
                    for tt in range(4):
                        nc.tensor.transpose(sb_[:, tt * HID:(tt + 1) * HID],
                                            sus[:, tt * P:(tt + 1) * P],
                                            K.ident[:HID, :HID])
                    tsb = sp.tile([P, 256], F32, name=f"{tag}_tsb", tag="tsb")
                    nc.vector.tensor_add(tsb[:], sb_[:],
                                         ag[:, g * 256:(g + 1) * 256])
                    # LayerNorm over 64-feature groups, row-major
                    t3 = tsb[:].rearrange("p (n d) -> p n d", d=HID)
                    mean = sp.tile([P, 4], F32, name=f"{tag}_mean", tag="mn")
                    nc.vector.tensor_reduce(mean[:], t3, axis=AX.X, op=OP.add)
                    nc.vector.tensor_scalar_mul(mean[:], mean[:], 1.0 / HID)
                    cen = sp.tile([P, 256], F32, name=f"{tag}_cen", tag="cn")
                    nc.vector.tensor_tensor(
                        cen[:].rearrange("p (n d) -> p n d", d=HID), t3,
                        _bcast_mid(mean[:], 4, HID), op=OP.subtract)
                    c3 = cen[:].rearrange("p (n d) -> p n d", d=HID)
                    sqf = sp.tile([P, 256], F32, name=f"{tag}_sqf", tag="sq")
                    nc.vector.tensor_mul(
                        sqf[:].rearrange("p (n d) -> p n d", d=HID), c3, c3)
                    ssq = sp.tile([P, 4], F32, name=f"{tag}_ssq", tag="ssq")
                    nc.vector.tensor_reduce(
                        ssq[:], sqf[:].rearrange("p (n d) -> p n d", d=HID),
                        axis=AX.X, op=OP.add)
                    sdv = sp.tile([P, 4], F32, name=f"{tag}_sdv", tag="sdv")
                    nc.scalar.activation(sdv[:], ssq[:], AF.Sqrt,
                                         bias=K.epsP[:, 0:1],
                                         scale=1.0 / HID)
                    rcp = sp.tile([P, 4], F32, name=f"{tag}_rcp", tag="rcp")
                    nc.vector.reciprocal(rcp[:], sdv[:])
                    nc.vector.tensor_tensor(c3, c3, _bcast_mid(rcp[:], 4, HID),
                                            op=OP.mult)
                    nc.vector.tensor_tensor(c3, c3,
                                            _bcast_row(K.lng[li][:], 4, HID),
                                            op=OP.mult)
                    nc.vector.tensor_tensor(c3, c3,
                                            _bcast_row(K.lnb[li][:], 4, HID),
                                            op=OP.add)
                    sgm = sp.tile([P, 256], BF16, name=f"{tag}_psg",
                                  tag="psg")
                    nc.scalar.activation(sgm[:], cen[:], AF.Sigmoid)
                    slu = sp.tile([P, 256], F32, name=f"{tag}_slu", tag="sl")
                    nc.vector.tensor_mul(slu[:], cen[:], sgm[:])
                    nc.vector.tensor_add(ob[:, g * 256:(g + 1) * 256], slu[:],
                                         et[:, g * 256:(g + 1) * 256])
                nc.sync.dma_start(
                    out=t_rearr(out_tab[ds(it * P, BLK), :]),
                    in_=ob[:].rearrange("p (n d) -> p n d", d=HID))

    # ------------------------------------------------------------------
    # node EGC (gates + chunked AllReduce + replicated post)
    # ------------------------------------------------------------------
    def node_egc(li, lslot, x_in, x_out, e_tab, tag):
        aq = aggr_n[lslot]
        ar_o = ar_out[lslot]
        for q in range(NQ):
            zero_rows(aq[q], NQR)
        for q in range(NQ):
            nidx = {"gd": ins[f"n_gxd{q}"], "gs": ins[f"n_gxs{q}"],
                    "sc": ins[f"n_sc{q}"], "sd": ins[f"n_sd{q}"],
                    "sdT": ins[f"n_sdT{q}"]}
            gate_phase(li, LNQ[q] // P, nidx, x_in, x_in, None, e_tab,
                       ins[f"n_ge{q}"], aq[q], NQR, f"{tag}g{q}")
            nc.gpsimd.collective_compute(
                "AllReduce", OP.add, replica_groups=RG,
                ins=[aq[q][:NQR, :]],
                outs=[ar_o[q * NQR:(q + 1) * NQR, :]])
        post_phase(li, NP_, x_in, ar_o, x_out, f"{tag}p")

    # ------------------------------------------------------------------
    # readout
    # ------------------------------------------------------------------
    def readout(x_fin):
        nt = NP_ // BLK
        with tc.tile_pool(name="ro_sb", bufs=3) as sp, \
             tc.tile_pool(name="ro_acc", bufs=1) as ac, \
             tc.tile_pool(name="ro_ps", bufs=2, space="PSUM") as pp:
            accs = []
            for h in range(GH):
                a = ac.tile([P, HID + 1], F32, name=f"ro_acc{h}")
                nc.gpsimd.memset(a[:], 0.0)
                accs.append(a)
            with tc.For_i(0, nt * SUB, SUB, staggered_reset=True) as it:
                xt = sp.tile([P, SUB * HID], BF16, name="ro_xt", tag="xt")
                nc.sync.dma_start(
                    out=xt[:].rearrange("p (n d) -> p n d", d=HID),
                    in_=t_rearr(x_fin[ds(it * P, BLK), :]))
                bt = sp.tile([P, SUB], F32, name="ro_bt", tag="bt")
                nc.sync.dma_start(out=bt[:], in_=ins["batchf"][:, ds(it, SUB)])
                for j in range(SUB):
                    xa = sp.tile([P, HID + 1], BF16, name="ro_xa", tag="xa")
                    nc.vector.tensor_copy(xa[:, :HID],
                                          xt[:, j * HID:(j + 1) * HID])
                    nc.gpsimd.memset(xa[:, HID:], 1.0)
                    M = sp.tile([P, GH * P], BF16, name="ro_M", tag="M")
                    nc.vector.tensor_tensor(
                        M[:], K.iotaf[:],
                        bt[:, j:j + 1].to_broadcast([P, GH * P]),
                        op=OP.is_equal)
                    for h in range(GH):
                        po = pp.tile([P, HID + 1], F32, name="ro_po")
                        nc.tensor.matmul(po[:], lhsT=M[:, h * P:(h + 1) * P],
                                         rhs=xa[:], start=True, stop=True)
                        nc.vector.tensor_add(accs[h][:], accs[h][:], po[:])
            out_sb = sp.tile([P, GH], F32, name="ro_out")
            for h in range(GH):
                cnt = sp.tile([P, 1], F32, name="ro_cnt", tag="cnt")
                nc.vector.tensor_scalar_max(cnt[:], accs[h][:, HID:], 1.0)
                rc = sp.tile([P, 1], F32, name="ro_rc", tag="rc")
                nc.vector.reciprocal(rc[:], cnt[:])
                pool = sp.tile([P, HID], BF16, name="ro_pool", tag="pool")
                nc.vector.tensor_tensor(pool[:], accs[h][:, :HID],
                                        rc[:].to_broadcast([P, HID]),
                                        op=OP.mult)
                pt = pp.tile([HID, P], BF16, name="ro_pt")
                nc.tensor.transpose(pt[:], pool[:], K.ident[:])
                pts = sp.tile([HID, P], BF16, name="ro_pts", tag="pts")
                nc.vector.tensor_copy(pts[:], pt[:])
                fc = pp.tile([HID, P], F32, name="ro_fc")
                nc.tensor.matmul(fc[:], lhsT=K.Wfc[:], rhs=pts[:],
                                 start=True, stop=True)
                zf = sp.tile([HID, P], F32, name="ro_zf", tag="zf")
                nc.vector.tensor_scalar(zf[:], fc[:], K.bfc[:, 0:1], None,
                                        op0=OP.add)
                sgf = sp.tile([HID, P], BF16, name="ro_sgf", tag="sgf")
                nc.scalar.activation(sgf[:], zf[:], AF.Sigmoid)
                hT = sp.tile([HID, P], BF16, name="ro_hT", tag="hT")
                nc.vector.tensor_mul(hT[:], zf[:], sgf[:])
                oo = pp.tile([P, 1], F32, name="ro_oo")
                nc.tensor.matmul(oo[:], lhsT=hT[:], rhs=K.Wout[:],
                                 start=True, stop=True)
                nc.scalar.activation(out_sb[:, h:h + 1], oo[:], AF.Identity,
                                     bias=K.boutP[:, 0:1])
            for h in range(GH):
                n = min(P, G - h * P)
                if n > 0:
                    nc.sync.dma_start(out=outs["out"][ds(h * P, n), None],
                                      in_=out_sb[:n, h:h + 1])

    # ------------------------------------------------------------------
    # program
    # ------------------------------------------------------------------
    x_emb()
    emb_rbf(ins["edist"], ESP, K.cent_e, meta["gam_e"], K.Wed,
            K.bemb[:, 1:2], NE + 1, e_bufs[0], None, "ee")
    emb_rbf(ins["angp"], LT, K.cent_a, meta["gam_a"], K.Wan,
            K.bemb[:, 2:3], NE + 2, None, aT, "ae")

    tidx = {k: ins["t_" + k] for k in ["gd", "gs", "sc", "sd", "sdT"]}
    send_a2a(0, e_bufs[0])
    for l in range(NLAY):
        zero_rows(aggr_e[l], ESP)
        gate_phase(2 * l, LT // P, tidx, e_bufs[l], recv_b[l], aT, None,
                   None, aggr_e[l], ES, f"eg{l}")
        post_phase(2 * l, ESP, e_bufs[l], aggr_e[l], e_bufs[l + 1],
                   f"ep{l}")
        if l + 1 < NLAY:
            send_a2a(l + 1, e_bufs[l + 1])
        node_egc(2 * l + 1, l, x_bufs[l], x_bufs[l + 1], e_bufs[l + 1],
                 f"na{l}")
    for gg in range(NLAY):
        node_egc(2 * NLAY + gg, NLAY + gg, x_bufs[NLAY + gg],
                 x_bufs[NLAY + gg + 1], e_bufs[NLAY], f"ng{gg}")

    readout(x_bufs[2 * NLAY])

    if meta.get("dbg"):
        for nm, tl in [("dbg_e0", e_bufs[0]), ("dbg_e1", e_bufs[1]),
                       ("dbg_x0", x_bufs[0]), ("dbg_x1", x_bufs[1]),
                       ("dbg_ag", aggr_e[0]), ("dbg_rv", recv_b[0]), ("dbg_sd", send_b[0]),
                       ("dbg_ar", ar_out[0]), ("dbg_xf", x_bufs[2 * NLAY])]:
            if nm in outs:
                nc.sync.dma_start(out=outs[nm], in_=tl[:])

    dram.release()
    K.pool.release()


# ----------------------------------------------------------------------------
# Runner
# ----------------------------------------------------------------------------

_DT = {np.dtype(np.float32): F32, np.dtype(BF): BF16,
       np.dtype(np.int32): I32}


def build_nc(meta, in_map0):
    C = meta["C"]
    nc = bacc.Bacc("TRN2", target_bir_lowering=False, debug=False,
                   num_devices=C)
    ins = {}
    for k, v in in_map0.items():
        t = nc.dram_tensor(k, list(v.shape), _DT[np.dtype(v.dtype)],
                           kind="ExternalInput")
        ins[k] = t[:]
    out_t = nc.dram_tensor("out", [meta["G"]], F32, kind="ExternalOutput")
    outs = {"out": out_t[:]}
    if meta.get("dbg"):
        ESP, NP_, NQR = meta["ESP"], meta["NP"], meta["NQR"]
        CP = meta["C"] * meta["PADM"]
        for nm, shp in [("dbg_e0", [ESP, HID]), ("dbg_e1", [ESP, HID]),
                        ("dbg_x0", [NP_, HID]), ("dbg_x1", [NP_, HID]),
                        ("dbg_ag", [ESP + P, HID]), ("dbg_rv", [CP, HID]), ("dbg_sd", [CP, HID]),
                        ("dbg_ar", [NP_, HID]), ("dbg_xf", [NP_, HID])]:
            outs[nm] = nc.dram_tensor(nm, shp, BF16, kind="ExternalOutput")[:]
    with tile.TileContext(nc) as tc:
        build(tc, outs, ins, meta)
    nc.compile()
    return nc


def kernel(**inputs):
    from concourse import bass_utils
    meta, in_maps = prep(inputs, C=8, BLK=4096)
    nc = build_nc(meta, in_maps[0])
    res = bass_utils.run_bass_kernel_spmd(nc, in_maps,
                                          core_ids=list(range(meta["C"])))
    return np.asarray(res.results[0]["out"], dtype=np.float32)


if __name__ == "__main__":
    pass
